# revision 7
# baseline (speedup 1.0000x reference)
"""GAT backbone (3-layer GATConv + graph pooling + loc-MLP) on 8 Trainium2
NeuronCores.

Strategy: dst-sharded edges. Each core owns a contiguous range of 6272
destination nodes (49 node-tiles of 128). Edges (with self-loops) are sorted
by dst on the host and padded so every (core, node-tile) has exactly K
128-edge tiles. Per edge-tile the core gathers table rows [h'|es|ed] for the
edge sources via indirect DMA, builds the dst one-hot on-device (iota compare
+ PE transpose), computes softmax weights w = exp(leaky(es_src + ed_dst)),
and scatter-accumulates [w*h' | w] into PSUM with a one-hot matmul. The
normalized output is transformed (h_in @ [W|As_eff|Ad_eff]) into the next
layer's table, which is replicated across cores with an AllGather. Graph mean
pool is a per-shard matmul with a host-built one-hot, AllReduce-summed across
cores. The loc-MLP is computed redundantly on every core.
"""
import numpy as np

# ---------------------------------------------------------------------------
# runtime patch: this walrus build accepts at most ONE sync-wait command per
# instruction; Tile attaches several. Split extras into single-wait NOPs.
# ---------------------------------------------------------------------------
_PATCHED = [False]


def _install_tile_patch():
    if _PATCHED[0]:
        return
    import concourse.mybir as mybir
    from concourse.tile import TileContext
    from concourse.vector_clock import ScopedClock

    ctr = [0]

    def _split(insts):
        new = []
        for inst in insts:
            si = getattr(inst, "sync_info", None)
            try:
                ow = si.on_wait if si is not None else None
            except Exception:
                ow = None
            if ow is not None and len(ow) > 1:
                waits = list(ow)
                for w in waits[:-1]:
                    ctr[0] += 1
                    nop = mybir.InstNoOp(name=f"wsplit-{ctr[0]}", ins=[], outs=[])
                    nop.engine = inst.engine
                    nop.sync_info = mybir.SyncInfo(on_wait=[w], on_update=[])
                    new.append(nop)
                si.on_wait = waits[-1:]
            new.append(inst)
        insts[:] = new

    orig_lower = TileContext._lower_ordered_insts

    def patched_lower(self, ordered):
        for insts in ordered.values():
            _split(insts)
        return orig_lower(self, ordered)

    def patched_drain(self, tick_clock, wait_clock):
        drain_inst = self.nc.sync.drain()
        wait_clock.add_sem_waits(
            drain_inst.ins, ScopedClock({None: tick_clock.global_clock})
        )
        si = drain_inst.ins.sync_info
        if si is not None and si.on_wait and len(si.on_wait) > 1:
            waits = list(si.on_wait)
            si.on_wait = waits[:1]
            for w in waits[1:]:
                extra = self.nc.sync.drain()
                esi = extra.ins.sync_info
                if esi is None:
                    extra.ins.sync_info = mybir.SyncInfo(on_wait=[w], on_update=[])
                else:
                    esi.on_wait = [w]
        self.nc.all_engine_barrier()
        assert self.sems is not None
        popped = self.nc._tile_sem_poison_stack.pop()
        assert popped is self._sem_poison
        self.nc.clear_and_free_semaphores(list(self.sems.allocated().values()))
        self.nc.all_engine_barrier()

    TileContext._lower_ordered_insts = patched_lower
    TileContext._drain_and_barrier = patched_drain
    _PATCHED[0] = True


# ---------------------------------------------------------------------------
# problem constants (hardcoded per contract)
# ---------------------------------------------------------------------------
N_NODES = 50000
N_EDGES = 800000
N_GRAPHS = 64
N_LOCS = 50
HEADS = 3
OPH = 43                    # out per head
MID = HEADS * OPH           # 129
GH = 128                    # gat hidden (layer 2 out)
MLPH = 256
NEG = 0.2
N_CORES = 8
SHARD = 6272                # 49 * 128 dst nodes per core
NT = SHARD // 128           # 49 node-tiles
NPAD = SHARD * N_CORES      # 50176
ZROW = NPAD                 # zero row index
TW = 136                    # table row width (f32): h'(<=129)|es|ed|pad

_CACHE = {}


def _host_prep(x, loc, edge_index, batch, W0, as0, ad0, b0, W1, as1, ad1, b1,
               W2, as2, ad2, b2, Wl1, bl1, Wl2, bl2):
    f32 = np.float32
    src = np.concatenate([edge_index[0], np.arange(N_NODES, dtype=np.int64)])
    dst = np.concatenate([edge_index[1], np.arange(N_NODES, dtype=np.int64)])

    # group edges per (core, node-tile)
    order = np.argsort(dst, kind="stable")
    src = src[order].astype(np.int64)
    dst = dst[order].astype(np.int64)
    tile_of = dst // 128                     # global node-tile id, 0..391
    # counts per global tile (node-tiles beyond 50000 have 0)
    n_tiles_total = NPAD // 128              # 392
    counts = np.bincount(tile_of, minlength=n_tiles_total)
    K = int(np.ceil(counts.max() / 128))
    cpt = counts.reshape(N_CORES, NT)
    K_nt = tuple(int(np.ceil(cpt[:, t].max() / 128)) for t in range(NT))
    starts = np.zeros(n_tiles_total + 1, np.int64)
    np.cumsum(counts, out=starts[1:])

    idx_all = np.full((N_CORES, NT, K * 128), ZROW, np.int32)
    rel_all = np.zeros((N_CORES, NT, K * 128), np.float32)
    for g in range(n_tiles_total):
        c, t = divmod(g, NT)
        s, e = starts[g], starts[g + 1]
        cnt = e - s
        idx_all[c, t, :cnt] = src[s:e]
        rel_all[c, t, :cnt] = (dst[s:e] - g * 128).astype(np.float32)
    # device layout [NT, 128, K]: edge j of tile t at [t, j%128, j//128]
    idx_dev = idx_all.reshape(N_CORES, NT, K, 128).transpose(0, 1, 3, 2).copy()
    rel_dev = rel_all.reshape(N_CORES, NT, K, 128).transpose(0, 1, 3, 2).copy()

    # pooling one-hot per core [NT, 128, 64] and counts
    bp = np.zeros((N_CORES, NT, 128, N_GRAPHS), f32)
    node = np.arange(NPAD)
    valid = node < N_NODES
    gid = np.where(valid, batch[np.minimum(node, N_NODES - 1)], 0)
    onehot = np.zeros((NPAD, N_GRAPHS), f32)
    onehot[valid, gid[valid]] = 1.0
    bp[:] = onehot.reshape(N_CORES, NT, 128, N_GRAPHS)
    cnt = np.bincount(batch, minlength=N_GRAPHS).astype(f32)
    cntinv = (1.0 / np.maximum(cnt, 1.0)).reshape(N_GRAPHS, 1).astype(f32)

    # weight packing: rhs_l = [W_l | W_l@Amat_s | W_l@Amat_d]
    def amat(a):
        h, o = a.shape
        m = np.zeros((h * o, h), f32)
        for i in range(h):
            m[i * o:(i + 1) * o, i] = a[i]
        return m

    rhs0 = np.concatenate([W0, W0 @ amat(as0), W0 @ amat(ad0)], axis=1).astype(f32)
    rhs1 = np.concatenate([W1, W1 @ amat(as1), W1 @ amat(ad1)], axis=1).astype(f32)
    rhs2 = np.concatenate([W2, W2 @ amat(as2), W2 @ amat(ad2)], axis=1).astype(f32)

    xT = np.zeros((6, NPAD), f32)
    xT[:, :N_NODES] = np.asarray(x, f32).T
    xT_own = xT.reshape(6, N_CORES, SHARD).transpose(1, 0, 2).copy()

    bb0 = np.tile(np.asarray(b0, f32)[None, :], (128, 1))
    bb1 = np.tile(np.asarray(b1, f32)[None, :], (128, 1))
    bb2 = np.tile(np.asarray(b2, f32)[None, :], (128, 1))

    locT = np.asarray(loc, f32).reshape(N_GRAPHS * N_LOCS, 2).T.copy()
    ident = np.eye(128, dtype=f32)

    common = dict(
        xT=xT, rhs0=rhs0, rhs1=rhs1, rhs2=rhs2,
        bb0=bb0, bb1=bb1, bb2=bb2, cntinv=cntinv,
        locT=locT, wl1=np.asarray(Wl1, f32), bl1c=np.asarray(bl1, f32).reshape(-1, 1),
        wl2=np.asarray(Wl2, f32), bl2c=np.asarray(bl2, f32).reshape(-1, 1),
        ident=ident,
    )
    in_maps = []
    for c in range(N_CORES):
        m = dict(common)
        m["idx"] = idx_dev[c]
        m["rel"] = rel_dev[c]
        m["bpool"] = bp[c]
        m["xTo"] = xT_own[c]
        in_maps.append(m)
    return in_maps, (K, K_nt)


def _build(Kinfo):
    K, K_nt = Kinfo
    _install_tile_patch()
    import concourse.bass as bass
    import concourse.mybir as mybir
    import concourse.tile as tile

    f32 = mybir.dt.float32
    i32 = mybir.dt.int32
    AF = mybir.ActivationFunctionType

    nc = bass.Bass(num_devices=N_CORES)

    inp = {}
    for name, shape, dt in [
        ("xT", [6, NPAD], f32), ("xTo", [6, SHARD], f32),
        ("idx", [NT, 128, K], i32), ("rel", [NT, 128, K], f32),
        ("bpool", [NT, 128, N_GRAPHS], f32), ("cntinv", [N_GRAPHS, 1], f32),
        ("rhs0", [6, MID + 6], f32), ("rhs1", [MID, MID + 6], f32),
        ("rhs2", [MID, GH + 2], f32),
        ("bb0", [128, MID], f32), ("bb1", [128, MID], f32), ("bb2", [128, GH], f32),
        ("locT", [2, N_GRAPHS * N_LOCS], f32), ("wl1", [2, MLPH], f32),
        ("bl1c", [MLPH, 1], f32), ("wl2", [MLPH, GH], f32), ("bl2c", [GH, 1], f32),
        ("ident", [128, 128], f32),
    ]:
        inp[name] = nc.dram_tensor(name, shape, dt, kind="ExternalInput")

    out = nc.dram_tensor("out", [N_GRAPHS, GH * 2], f32, kind="ExternalOutput")

    # tables
    T = [
        nc.dram_tensor("T0", [NPAD + 1, TW], f32, kind="Internal"),
        nc.dram_tensor("T1", [NPAD + 1, TW], f32, kind="Internal",
                       addr_space="Shared"),
        nc.dram_tensor("T2", [NPAD + 1, TW], f32, kind="Internal",
                       addr_space="Shared"),
    ]
    Tsh = [
        nc.dram_tensor("Tsh1", [SHARD, TW], f32, kind="Internal"),
        nc.dram_tensor("Tsh2", [SHARD, TW], f32, kind="Internal"),
    ]
    s_in = nc.dram_tensor("s_in", [N_GRAPHS, GH], f32, kind="Internal")
    s_out = nc.dram_tensor("s_out", [N_GRAPHS, GH], f32, kind="Internal",
                           addr_space="Shared")

    # per-layer config: (feat width F, heads H) — rhs width = F + H
    LCFG = [(MID, HEADS), (MID, HEADS), (GH, 1)]
    RG = [list(range(N_CORES))]

    with tile.TileContext(nc) as tc:
        with tc.tile_pool(name="const", bufs=1) as cp, \
             tc.tile_pool(name="sb", bufs=4) as sb, \
             tc.tile_pool(name="ed", bufs=1) as edp, \
             tc.tile_pool(name="mlp", bufs=1) as mlppool, \
             tc.tile_pool(name="ps", bufs=4, space="PSUM") as ps, \
             tc.tile_pool(name="psacc", bufs=2, space="PSUM") as psacc, \
             tc.tile_pool(name="pspool", bufs=1, space="PSUM") as pspool:

            ident = cp.tile([128, 128], f32)
            nc.sync.dma_start(ident[:], inp["ident"][:, :])
            iota_row = cp.tile([128, 128], f32)
            nc.gpsimd.iota(iota_row[:], pattern=[[1, 128]], base=0,
                        channel_multiplier=0,
                        allow_small_or_imprecise_dtypes=True)

            rhs_sb = []      # layer 0: [6, 135]
            t0r = cp.tile([6, MID + 6], f32, tag="rhs0")
            nc.sync.dma_start(t0r[:], inp["rhs0"][:, :])
            rhs_sb.append(t0r)
            rhs_a, rhs_b = {}, {}   # layers 1,2: split [128,W] + [1,W]
            for l, nm in [(1, "rhs1"), (2, "rhs2")]:
                w = inp[nm].shape[1]
                ta = cp.tile([128, w], f32, tag=f"rhsa{l}")
                nc.sync.dma_start(ta[:], inp[nm][0:128, :])
                tb = cp.tile([1, w], f32, tag=f"rhsb{l}")
                nc.sync.dma_start(tb[:], inp[nm][128:129, :])
                rhs_a[l], rhs_b[l] = ta, tb
            bb_sb = []
            for l, nm in enumerate(["bb0", "bb1", "bb2"]):
                t = cp.tile(list(inp[nm].shape), f32, tag=f"bb{l}")
                nc.sync.dma_start(t[:], inp[nm][:, :])
                bb_sb.append(t)

            # ed values for own shard, per layer: [128, NT, H]
            ed_all = [edp.tile([128, NT, 3], f32, tag=f"edall{l}", name=f"edall{l}") for l in range(3)]

            # ---------------- stage A: build T0 for all nodes ----------------
            for j in range(NPAD // 128):
                xs = sb.tile([6, 128], f32, tag="xs")
                nc.sync.dma_start(xs[:], inp["xT"][:, j * 128:(j + 1) * 128])
                ptab = ps.tile([128, MID + 6], f32, space="PSUM", tag="pscr")
                nc.tensor.matmul(ptab[:], lhsT=xs[:], rhs=rhs_sb[0][:],
                                 start=True, stop=True)
                stg = sb.tile([128, TW], f32, tag="stg0")
                nc.vector.memset(stg[:, MID + 6:], 0.0)
                nc.vector.tensor_copy(stg[:, :MID + 6], ptab[:])
                nc.sync.dma_start(T[0][j * 128:(j + 1) * 128, :], stg[:])
            zz = sb.tile([1, TW], f32, tag="zrow")
            nc.vector.memset(zz[:], 0.0)
            nc.vector.memset(zz[:, MID:MID + 3], -1000.0)
            nc.sync.dma_start(T[0][ZROW:ZROW + 1, :], zz[:])
            # ed0 for own shard (from per-core xTo input)
            for t in range(NT):
                xo = sb.tile([6, 128], f32, tag="xs")
                nc.sync.dma_start(xo[:], inp["xTo"][:, t * 128:(t + 1) * 128])
                pe0 = ps.tile([128, 6], f32, space="PSUM", tag="pscr")
                nc.tensor.matmul(pe0[:], lhsT=xo[:], rhs=rhs_sb[0][:, MID:MID + 6],
                                 start=True, stop=True)
                nc.vector.tensor_copy(ed_all[0][:, t, :], pe0[:, 3:6])

            # ---------------- loc MLP (independent) ----------------
            locT = mlppool.tile([2, N_GRAPHS * N_LOCS], f32)
            nc.sync.dma_start(locT[:], inp["locT"][:, :])
            wl1 = mlppool.tile([2, MLPH], f32)
            nc.sync.dma_start(wl1[:], inp["wl1"][:, :])
            bl1c = mlppool.tile([128, 2], f32)
            nc.sync.dma_start(bl1c[:], inp["bl1c"][:, 0:1].rearrange("(h p) o -> p (h o)", p=128))
            wl2a = mlppool.tile([128, GH], f32)
            nc.sync.dma_start(wl2a[:], inp["wl2"][0:128, :])
            wl2b = mlppool.tile([128, GH], f32)
            nc.sync.dma_start(wl2b[:], inp["wl2"][128:256, :])
            bl2c = mlppool.tile([GH, 1], f32)
            nc.sync.dma_start(bl2c[:], inp["bl2c"][:, :])
            mid_sb = [mlppool.tile([128, N_GRAPHS * N_LOCS], f32, tag=f"mid{h}", name=f"mid{h}")
                      for h in range(2)]
            CH = 400
            nch = (N_GRAPHS * N_LOCS) // CH
            for h in range(2):
                for c in range(nch):
                    pm = ps.tile([128, CH], f32, space="PSUM", tag="pscr")
                    nc.tensor.matmul(pm[:], lhsT=wl1[:, h * 128:(h + 1) * 128],
                                     rhs=locT[:, c * CH:(c + 1) * CH],
                                     start=True, stop=True)
                    nc.scalar.activation(mid_sb[h][:, c * CH:(c + 1) * CH], pm[:],
                                         AF.Tanh, bias=bl1c[:, h:h + 1])
            lpT = mlppool.tile([128, N_GRAPHS], f32)
            for c in range(nch):
                po = ps.tile([128, CH], f32, space="PSUM", tag="pscr")
                nc.tensor.matmul(po[:], lhsT=wl2a[:],
                                 rhs=mid_sb[0][:, c * CH:(c + 1) * CH],
                                 start=True, stop=False)
                nc.tensor.matmul(po[:], lhsT=wl2b[:],
                                 rhs=mid_sb[1][:, c * CH:(c + 1) * CH],
                                 start=False, stop=True)
                ng = CH // N_LOCS
                nc.vector.reduce_sum(
                    lpT[:, c * ng:(c + 1) * ng],
                    po[:].rearrange("p (g l) -> p g l", l=N_LOCS),
                    axis=mybir.AxisListType.X)
            lpT2 = mlppool.tile([128, N_GRAPHS], f32)
            nc.vector.tensor_scalar(lpT2[:], lpT[:], 1.0 / N_LOCS, bl2c[:],
                                    mybir.AluOpType.mult, mybir.AluOpType.add)
            plp = ps.tile([N_GRAPHS, 128], f32, space="PSUM", tag="pscr")
            nc.tensor.matmul(plp[:], lhsT=lpT2[:], rhs=ident[:],
                             start=True, stop=True)
            out_sb = mlppool.tile([N_GRAPHS, 2 * GH], f32)
            nc.vector.tensor_copy(out_sb[:, GH:], plp[:])

            # ---------------- GAT layers ----------------
            psum_S = pspool.tile([N_GRAPHS, GH], f32, space="PSUM")

            for l in range(3):
                F, H = LCFG[l]
                RW = F + H                       # rhs/scatter width
                tab = T[l]
                for nt in range(NT):
                    idxs = sb.tile([128, K], i32, tag="idxs")
                    nc.sync.dma_start(idxs[:], inp["idx"][nt, :, :])
                    rels = sb.tile([128, K], f32, tag="rels")
                    nc.sync.dma_start(rels[:], inp["rel"][nt, :, :])
                    acc = psacc.tile([128, RW], f32, space="PSUM", tag="acc")
                    Kt = K_nt[nt]
                    for t in range(Kt):
                        g = sb.tile([128, TW], f32, tag="g")
                        nc.gpsimd.indirect_dma_start(
                            out=g[:], out_offset=None, in_=tab[:, :],
                            in_offset=bass.IndirectOffsetOnAxis(
                                ap=idxs[:, t:t + 1], axis=0))
                        pedge = sb.tile([128, 128], f32, tag="pedge")
                        nc.vector.tensor_tensor(
                            out=pedge[:], in0=rels[:, t:t + 1].to_broadcast([128, 128]),
                            in1=iota_row[:], op=mybir.AluOpType.is_equal)
                        ptr = ps.tile([128, 128], f32, space="PSUM", tag="pscr")
                        nc.tensor.matmul(ptr[:], lhsT=pedge[:], rhs=ident[:],
                                         start=True, stop=True)
                        pdst = sb.tile([128, 128], f32, tag="pdst")
                        nc.vector.tensor_copy(pdst[:], ptr[:])
                        pede = ps.tile([128, 3], f32, space="PSUM", tag="pscr")
                        nc.tensor.matmul(pede[:, :H], lhsT=pdst[:],
                                         rhs=ed_all[l][:, nt, :H],
                                         start=True, stop=True)
                        e = sb.tile([128, 3], f32, tag="e")
                        nc.vector.tensor_add(e[:, :H], g[:, F:F + H], pede[:, :H])
                        lk = sb.tile([128, 3], f32, tag="lk")
                        nc.scalar.activation(lk[:, :H], e[:, :H], AF.Prelu, alpha=NEG)
                        w = sb.tile([128, 3], f32, tag="w")
                        nc.scalar.activation(w[:, :H], lk[:, :H], AF.Exp)
                        rhs_t = sb.tile([128, RW], f32, tag="rhs_t")
                        oph = F // H
                        for hh in range(H):
                            nc.vector.tensor_scalar_mul(
                                rhs_t[:, hh * oph:(hh + 1) * oph],
                                g[:, hh * oph:(hh + 1) * oph], w[:, hh:hh + 1])
                        nc.vector.tensor_copy(rhs_t[:, F:F + H], w[:, :H])
                        nc.tensor.matmul(acc[:], lhsT=pedge[:], rhs=rhs_t[:],
                                         start=(t == 0), stop=(t == Kt - 1))
                    # epilogue for this node-tile
                    zc = sb.tile([128, 3], f32, tag="zc")
                    nc.vector.tensor_scalar_max(zc[:, :H], acc[:, F:F + H], 1e-30)
                    zr = sb.tile([128, 3], f32, tag="zr")
                    nc.vector.reciprocal(zr[:, :H], zc[:, :H])
                    u = sb.tile([128, F], f32, tag="u")
                    oph = F // H
                    for hh in range(H):
                        nc.vector.tensor_scalar_mul(
                            u[:, hh * oph:(hh + 1) * oph],
                            acc[:, hh * oph:(hh + 1) * oph], zr[:, hh:hh + 1])
                    ob = sb.tile([128, F], f32, tag="ob")
                    nc.vector.tensor_add(ob[:], u[:], bb_sb[l][:, :F])
                    if l < 2:
                        hin = sb.tile([128, F], f32, tag="hin")
                        nc.scalar.activation(hin[:], ob[:], AF.Prelu, alpha=NEG)
                        # transpose hin -> [F, 128] in two pieces
                        ph1 = ps.tile([128, 128], f32, space="PSUM", tag="pscr")
                        nc.tensor.matmul(ph1[:], lhsT=hin[:, 0:128], rhs=ident[:],
                                         start=True, stop=True)
                        hTa = sb.tile([128, 128], f32, tag="hTa")
                        nc.vector.tensor_copy(hTa[:], ph1[:])
                        ph2 = ps.tile([1, 128], f32, space="PSUM", tag="pscr")
                        nc.tensor.matmul(ph2[:], lhsT=hin[:, 128:129], rhs=ident[:],
                                         start=True, stop=True)
                        hTb = sb.tile([1, 128], f32, tag="hTb")
                        nc.vector.tensor_copy(hTb[:], ph2[:])
                        nF2, nH2 = LCFG[l + 1]
                        ptab = ps.tile([128, nF2 + 2 * nH2], f32, space="PSUM",
                                       tag="pscr")
                        nc.tensor.matmul(ptab[:], lhsT=hTa[:],
                                         rhs=rhs_a[l + 1][:],
                                         start=True, stop=False)
                        nc.tensor.matmul(ptab[:], lhsT=hTb[:],
                                         rhs=rhs_b[l + 1][:],
                                         start=False, stop=True)
                        stg = sb.tile([128, TW], f32, tag="stg")
                        nc.vector.memset(stg[:, nF2 + 2 * nH2:], 0.0)
                        nc.vector.tensor_copy(stg[:, :nF2 + 2 * nH2], ptab[:])
                        nc.sync.dma_start(
                            Tsh[l][nt * 128:(nt + 1) * 128, :], stg[:])
                        nc.vector.tensor_copy(
                            ed_all[l + 1][:, nt, :nH2],
                            ptab[:, nF2 + nH2:nF2 + 2 * nH2])
                    else:
                        bpt = sb.tile([128, N_GRAPHS], f32, tag="bpt")
                        nc.sync.dma_start(bpt[:], inp["bpool"][nt, :, :])
                        nc.tensor.matmul(psum_S[:], lhsT=bpt[:], rhs=ob[:],
                                         start=(nt == 0), stop=(nt == NT - 1))
                if l < 2:
                    # zero row of next table, then replicate shards
                    zz2 = sb.tile([1, TW], f32, tag="zrow")
                    nc.vector.memset(zz2[:], 0.0)
                    nF2, nH2 = LCFG[l + 1]
                    nc.vector.memset(zz2[:, nF2:nF2 + nH2], -1000.0)
                    nc.sync.dma_start(T[l + 1][ZROW:ZROW + 1, :], zz2[:])
                    nc.gpsimd.collective_compute(
                        "AllGather", mybir.AluOpType.bypass, replica_groups=RG,
                        ins=[Tsh[l][:, :]], outs=[T[l + 1][0:NPAD, :]])

            # pooling: AllReduce of per-shard sums, then divide by counts
            ssb = sb.tile([N_GRAPHS, GH], f32, tag="ssb")
            nc.vector.tensor_copy(ssb[:], psum_S[:])
            nc.sync.dma_start(s_in[:, :], ssb[:])
            nc.gpsimd.collective_compute(
                "AllReduce", mybir.AluOpType.add, replica_groups=RG,
                ins=[s_in[:, :]], outs=[s_out[:, :]])
            sfull = sb.tile([N_GRAPHS, GH], f32, tag="sfull")
            nc.sync.dma_start(sfull[:], s_out[:, :])
            civ = sb.tile([N_GRAPHS, 1], f32, tag="civ")
            nc.sync.dma_start(civ[:], inp["cntinv"][:, :])
            nc.vector.tensor_scalar_mul(out_sb[:, 0:GH], sfull[:], civ[:])
            nc.sync.dma_start(out[:, :], out_sb[:])

    return nc


def kernel(**inputs):
    key = "k"
    in_maps, Kinfo = _host_prep(**inputs)
    if key not in _CACHE or _CACHE[key][1] != Kinfo:
        nc = _build(Kinfo)
        _CACHE[key] = (nc, Kinfo)
    nc = _CACHE[key][0]
    from concourse.bass_utils import run_bass_kernel_spmd
    res = run_bass_kernel_spmd(nc, in_maps, core_ids=list(range(N_CORES)))
    return np.asarray(res.results[0]["out"])


# revision 8
# speedup vs baseline: 1.1251x; 1.1251x over previous
"""GAT backbone (3-layer GATConv + graph pooling + loc-MLP) on 8 Trainium2
NeuronCores.

Strategy: dst-sharded edges. Each core owns a contiguous range of 6272
destination nodes (49 node-tiles of 128). Edges (with self-loops) are sorted
by dst on the host and padded so every (core, node-tile) has exactly K
128-edge tiles. Per edge-tile the core gathers table rows [h'|es|ed] for the
edge sources via indirect DMA, builds the dst one-hot on-device (iota compare
+ PE transpose), computes softmax weights w = exp(leaky(es_src + ed_dst)),
and scatter-accumulates [w*h' | w] into PSUM with a one-hot matmul. The
normalized output is transformed (h_in @ [W|As_eff|Ad_eff]) into the next
layer's table, which is replicated across cores with an AllGather. Graph mean
pool is a per-shard matmul with a host-built one-hot, AllReduce-summed across
cores. The loc-MLP is computed redundantly on every core.
"""
import numpy as np

# ---------------------------------------------------------------------------
# runtime patch: this walrus build accepts at most ONE sync-wait command per
# instruction; Tile attaches several. Split extras into single-wait NOPs.
# ---------------------------------------------------------------------------
_PATCHED = [False]


def _install_tile_patch():
    if _PATCHED[0]:
        return
    import concourse.mybir as mybir
    from concourse.tile import TileContext
    from concourse.vector_clock import ScopedClock

    ctr = [0]

    def _split(insts):
        new = []
        for inst in insts:
            si = getattr(inst, "sync_info", None)
            try:
                ow = si.on_wait if si is not None else None
            except Exception:
                ow = None
            if ow is not None and len(ow) > 1:
                waits = list(ow)
                for w in waits[:-1]:
                    ctr[0] += 1
                    nop = mybir.InstNoOp(name=f"wsplit-{ctr[0]}", ins=[], outs=[])
                    nop.engine = inst.engine
                    nop.sync_info = mybir.SyncInfo(on_wait=[w], on_update=[])
                    new.append(nop)
                si.on_wait = waits[-1:]
            new.append(inst)
        insts[:] = new

    orig_lower = TileContext._lower_ordered_insts

    def patched_lower(self, ordered):
        for insts in ordered.values():
            _split(insts)
        return orig_lower(self, ordered)

    def patched_drain(self, tick_clock, wait_clock):
        drain_inst = self.nc.sync.drain()
        wait_clock.add_sem_waits(
            drain_inst.ins, ScopedClock({None: tick_clock.global_clock})
        )
        si = drain_inst.ins.sync_info
        if si is not None and si.on_wait and len(si.on_wait) > 1:
            waits = list(si.on_wait)
            si.on_wait = waits[:1]
            for w in waits[1:]:
                extra = self.nc.sync.drain()
                esi = extra.ins.sync_info
                if esi is None:
                    extra.ins.sync_info = mybir.SyncInfo(on_wait=[w], on_update=[])
                else:
                    esi.on_wait = [w]
        self.nc.all_engine_barrier()
        assert self.sems is not None
        popped = self.nc._tile_sem_poison_stack.pop()
        assert popped is self._sem_poison
        self.nc.clear_and_free_semaphores(list(self.sems.allocated().values()))
        self.nc.all_engine_barrier()

    TileContext._lower_ordered_insts = patched_lower
    TileContext._drain_and_barrier = patched_drain
    _PATCHED[0] = True


# ---------------------------------------------------------------------------
# problem constants (hardcoded per contract)
# ---------------------------------------------------------------------------
N_NODES = 50000
N_EDGES = 800000
N_GRAPHS = 64
N_LOCS = 50
HEADS = 3
OPH = 43                    # out per head
MID = HEADS * OPH           # 129
GH = 128                    # gat hidden (layer 2 out)
MLPH = 256
NEG = 0.2
N_CORES = 8
SHARD = 6272                # 49 * 128 dst nodes per core
NT = SHARD // 128           # 49 node-tiles
NPAD = SHARD * N_CORES      # 50176
ZROW = NPAD                 # zero row index
TW = 136                    # table row width (f32): h'(<=129)|es|ed|pad

_CACHE = {}


def _host_prep(x, loc, edge_index, batch, W0, as0, ad0, b0, W1, as1, ad1, b1,
               W2, as2, ad2, b2, Wl1, bl1, Wl2, bl2):
    f32 = np.float32
    src = np.concatenate([edge_index[0], np.arange(N_NODES, dtype=np.int64)])
    dst = np.concatenate([edge_index[1], np.arange(N_NODES, dtype=np.int64)])

    # group edges per (core, node-tile)
    order = np.argsort(dst, kind="stable")
    src = src[order].astype(np.int64)
    dst = dst[order].astype(np.int64)
    tile_of = dst // 128                     # global node-tile id, 0..391
    # counts per global tile (node-tiles beyond 50000 have 0)
    n_tiles_total = NPAD // 128              # 392
    counts = np.bincount(tile_of, minlength=n_tiles_total)
    K = int(np.ceil(counts.max() / 128))
    cpt = counts.reshape(N_CORES, NT)
    K_nt = tuple(int(np.ceil(cpt[:, t].max() / 128)) for t in range(NT))
    starts = np.zeros(n_tiles_total + 1, np.int64)
    np.cumsum(counts, out=starts[1:])

    idx_all = np.full((N_CORES, NT, K * 128), ZROW, np.int32)
    rel_all = np.zeros((N_CORES, NT, K * 128), np.float32)
    for g in range(n_tiles_total):
        c, t = divmod(g, NT)
        s, e = starts[g], starts[g + 1]
        cnt = e - s
        idx_all[c, t, :cnt] = src[s:e]
        rel_all[c, t, :cnt] = (dst[s:e] - g * 128).astype(np.float32)
    # device layout [NT, 128, K]: edge j of tile t at [t, j%128, j//128]
    idx_dev = idx_all.reshape(N_CORES, NT, K, 128).transpose(0, 1, 3, 2).copy()
    rel_dev = rel_all.reshape(N_CORES, NT, K, 128).transpose(0, 1, 3, 2).copy()
    # host-built P_dst one-hot per edge-tile, fp8: [NT, K, 128 dst, 128 edges]
    import ml_dtypes
    rel_i = rel_all.reshape(N_CORES, NT, K, 128).astype(np.int64)
    pdst_dev = np.zeros((N_CORES, NT, K, 128, 128), ml_dtypes.float8_e4m3)
    eye8 = np.eye(128, dtype=ml_dtypes.float8_e4m3)
    for c in range(N_CORES):
        for t in range(NT):
            for k in range(K):
                pdst_dev[c, t, k] = eye8[:, rel_i[c, t, k]]

    # pooling one-hot per core [NT, 128, 64] and counts
    bp = np.zeros((N_CORES, NT, 128, N_GRAPHS), f32)
    node = np.arange(NPAD)
    valid = node < N_NODES
    gid = np.where(valid, batch[np.minimum(node, N_NODES - 1)], 0)
    onehot = np.zeros((NPAD, N_GRAPHS), f32)
    onehot[valid, gid[valid]] = 1.0
    bp[:] = onehot.reshape(N_CORES, NT, 128, N_GRAPHS)
    cnt = np.bincount(batch, minlength=N_GRAPHS).astype(f32)
    cntinv = (1.0 / np.maximum(cnt, 1.0)).reshape(N_GRAPHS, 1).astype(f32)

    # weight packing: rhs_l = [W_l | W_l@Amat_s | W_l@Amat_d]
    def amat(a):
        h, o = a.shape
        m = np.zeros((h * o, h), f32)
        for i in range(h):
            m[i * o:(i + 1) * o, i] = a[i]
        return m

    rhs0 = np.concatenate([W0, W0 @ amat(as0), W0 @ amat(ad0)], axis=1).astype(f32)
    rhs1 = np.concatenate([W1, W1 @ amat(as1), W1 @ amat(ad1)], axis=1).astype(f32)
    rhs2 = np.concatenate([W2, W2 @ amat(as2), W2 @ amat(ad2)], axis=1).astype(f32)

    xT = np.zeros((6, NPAD), f32)
    xT[:, :N_NODES] = np.asarray(x, f32).T
    xT_own = xT.reshape(6, N_CORES, SHARD).transpose(1, 0, 2).copy()

    bb0 = np.tile(np.asarray(b0, f32)[None, :], (128, 1))
    bb1 = np.tile(np.asarray(b1, f32)[None, :], (128, 1))
    bb2 = np.tile(np.asarray(b2, f32)[None, :], (128, 1))

    locT = np.asarray(loc, f32).reshape(N_GRAPHS * N_LOCS, 2).T.copy()
    ident = np.eye(128, dtype=f32)

    common = dict(
        xT=xT, rhs0=rhs0, rhs1=rhs1, rhs2=rhs2,
        bb0=bb0, bb1=bb1, bb2=bb2, cntinv=cntinv,
        locT=locT, wl1=np.asarray(Wl1, f32), bl1c=np.asarray(bl1, f32).reshape(-1, 1),
        wl2=np.asarray(Wl2, f32), bl2c=np.asarray(bl2, f32).reshape(-1, 1),
        ident=ident,
    )
    in_maps = []
    for c in range(N_CORES):
        m = dict(common)
        m["idx"] = idx_dev[c]
        m["rel"] = rel_dev[c]
        m["bpool"] = bp[c]
        m["xTo"] = xT_own[c]
        m["pdst"] = pdst_dev[c]
        in_maps.append(m)
    return in_maps, (K, K_nt)


def _build(Kinfo):
    K, K_nt = Kinfo
    _install_tile_patch()
    import concourse.bass as bass
    import concourse.mybir as mybir
    import concourse.tile as tile

    f32 = mybir.dt.float32
    i32 = mybir.dt.int32
    AF = mybir.ActivationFunctionType

    nc = bass.Bass(num_devices=N_CORES)

    inp = {}
    for name, shape, dt in [
        ("xT", [6, NPAD], f32), ("xTo", [6, SHARD], f32),
        ("idx", [NT, 128, K], i32), ("rel", [NT, 128, K], f32),
        ("pdst", [NT, K, 128, 128], mybir.dt.float8e4),
        ("bpool", [NT, 128, N_GRAPHS], f32), ("cntinv", [N_GRAPHS, 1], f32),
        ("rhs0", [6, MID + 6], f32), ("rhs1", [MID, MID + 6], f32),
        ("rhs2", [MID, GH + 2], f32),
        ("bb0", [128, MID], f32), ("bb1", [128, MID], f32), ("bb2", [128, GH], f32),
        ("locT", [2, N_GRAPHS * N_LOCS], f32), ("wl1", [2, MLPH], f32),
        ("bl1c", [MLPH, 1], f32), ("wl2", [MLPH, GH], f32), ("bl2c", [GH, 1], f32),
        ("ident", [128, 128], f32),
    ]:
        inp[name] = nc.dram_tensor(name, shape, dt, kind="ExternalInput")

    out = nc.dram_tensor("out", [N_GRAPHS, GH * 2], f32, kind="ExternalOutput")

    # tables
    T = [
        nc.dram_tensor("T0", [NPAD + 1, TW], f32, kind="Internal"),
        nc.dram_tensor("T1", [NPAD + 1, TW], f32, kind="Internal",
                       addr_space="Shared"),
        nc.dram_tensor("T2", [NPAD + 1, TW], f32, kind="Internal",
                       addr_space="Shared"),
    ]
    Tsh = [
        nc.dram_tensor("Tsh1", [SHARD, TW], f32, kind="Internal"),
        nc.dram_tensor("Tsh2", [SHARD, TW], f32, kind="Internal"),
    ]
    s_in = nc.dram_tensor("s_in", [N_GRAPHS, GH], f32, kind="Internal")
    s_out = nc.dram_tensor("s_out", [N_GRAPHS, GH], f32, kind="Internal",
                           addr_space="Shared")

    # per-layer config: (feat width F, heads H) — rhs width = F + H
    LCFG = [(MID, HEADS), (MID, HEADS), (GH, 1)]
    RG = [list(range(N_CORES))]

    with tile.TileContext(nc) as tc:
        with tc.tile_pool(name="const", bufs=1) as cp, \
             tc.tile_pool(name="sb", bufs=4) as sb, \
             tc.tile_pool(name="ed", bufs=1) as edp, \
             tc.tile_pool(name="mlp", bufs=1) as mlppool, \
             tc.tile_pool(name="ps", bufs=4, space="PSUM") as ps, \
             tc.tile_pool(name="psacc", bufs=2, space="PSUM") as psacc, \
             tc.tile_pool(name="pspool", bufs=1, space="PSUM") as pspool:

            ident = cp.tile([128, 128], f32)
            nc.sync.dma_start(ident[:], inp["ident"][:, :])
            iota_row = cp.tile([128, 128], f32)
            nc.gpsimd.iota(iota_row[:], pattern=[[1, 128]], base=0,
                        channel_multiplier=0,
                        allow_small_or_imprecise_dtypes=True)

            rhs_sb = []      # layer 0: [6, 135]
            t0r = cp.tile([6, MID + 6], f32, tag="rhs0")
            nc.sync.dma_start(t0r[:], inp["rhs0"][:, :])
            rhs_sb.append(t0r)
            rhs_a, rhs_b = {}, {}   # layers 1,2: split [128,W] + [1,W]
            for l, nm in [(1, "rhs1"), (2, "rhs2")]:
                w = inp[nm].shape[1]
                ta = cp.tile([128, w], f32, tag=f"rhsa{l}")
                nc.sync.dma_start(ta[:], inp[nm][0:128, :])
                tb = cp.tile([1, w], f32, tag=f"rhsb{l}")
                nc.sync.dma_start(tb[:], inp[nm][128:129, :])
                rhs_a[l], rhs_b[l] = ta, tb
            bb_sb = []
            for l, nm in enumerate(["bb0", "bb1", "bb2"]):
                t = cp.tile(list(inp[nm].shape), f32, tag=f"bb{l}")
                nc.sync.dma_start(t[:], inp[nm][:, :])
                bb_sb.append(t)

            # ed values for own shard, per layer: [128, NT, H]
            ed_all = [edp.tile([128, NT, 3], mybir.dt.bfloat16, tag=f"edall{l}", name=f"edall{l}") for l in range(3)]

            # ---------------- stage A: build T0 for all nodes ----------------
            for j in range(NPAD // 128):
                xs = sb.tile([6, 128], f32, tag="xs")
                nc.sync.dma_start(xs[:], inp["xT"][:, j * 128:(j + 1) * 128])
                ptab = ps.tile([128, MID + 6], f32, space="PSUM", tag="pscr")
                nc.tensor.matmul(ptab[:], lhsT=xs[:], rhs=rhs_sb[0][:],
                                 start=True, stop=True)
                stg = sb.tile([128, TW], f32, tag="stg0")
                nc.vector.memset(stg[:, MID + 6:], 0.0)
                nc.vector.tensor_copy(stg[:, :MID + 6], ptab[:])
                nc.sync.dma_start(T[0][j * 128:(j + 1) * 128, :], stg[:])
            zz = sb.tile([1, TW], f32, tag="zrow")
            nc.vector.memset(zz[:], 0.0)
            nc.vector.memset(zz[:, MID:MID + 3], -1000.0)
            nc.sync.dma_start(T[0][ZROW:ZROW + 1, :], zz[:])
            # ed0 for own shard (from per-core xTo input)
            for t in range(NT):
                xo = sb.tile([6, 128], f32, tag="xs")
                nc.sync.dma_start(xo[:], inp["xTo"][:, t * 128:(t + 1) * 128])
                pe0 = ps.tile([128, 6], f32, space="PSUM", tag="pscr")
                nc.tensor.matmul(pe0[:], lhsT=xo[:], rhs=rhs_sb[0][:, MID:MID + 6],
                                 start=True, stop=True)
                nc.vector.tensor_copy(ed_all[0][:, t, :], pe0[:, 3:6])

            # ---------------- loc MLP (independent) ----------------
            locT = mlppool.tile([2, N_GRAPHS * N_LOCS], f32)
            nc.sync.dma_start(locT[:], inp["locT"][:, :])
            wl1 = mlppool.tile([2, MLPH], f32)
            nc.sync.dma_start(wl1[:], inp["wl1"][:, :])
            bl1c = mlppool.tile([128, 2], f32)
            nc.sync.dma_start(bl1c[:], inp["bl1c"][:, 0:1].rearrange("(h p) o -> p (h o)", p=128))
            wl2a = mlppool.tile([128, GH], f32)
            nc.sync.dma_start(wl2a[:], inp["wl2"][0:128, :])
            wl2b = mlppool.tile([128, GH], f32)
            nc.sync.dma_start(wl2b[:], inp["wl2"][128:256, :])
            bl2c = mlppool.tile([GH, 1], f32)
            nc.sync.dma_start(bl2c[:], inp["bl2c"][:, :])
            mid_sb = [mlppool.tile([128, N_GRAPHS * N_LOCS], f32, tag=f"mid{h}", name=f"mid{h}")
                      for h in range(2)]
            CH = 400
            nch = (N_GRAPHS * N_LOCS) // CH
            for h in range(2):
                for c in range(nch):
                    pm = ps.tile([128, CH], f32, space="PSUM", tag="pscr")
                    nc.tensor.matmul(pm[:], lhsT=wl1[:, h * 128:(h + 1) * 128],
                                     rhs=locT[:, c * CH:(c + 1) * CH],
                                     start=True, stop=True)
                    nc.scalar.activation(mid_sb[h][:, c * CH:(c + 1) * CH], pm[:],
                                         AF.Tanh, bias=bl1c[:, h:h + 1])
            lpT = mlppool.tile([128, N_GRAPHS], f32)
            for c in range(nch):
                po = ps.tile([128, CH], f32, space="PSUM", tag="pscr")
                nc.tensor.matmul(po[:], lhsT=wl2a[:],
                                 rhs=mid_sb[0][:, c * CH:(c + 1) * CH],
                                 start=True, stop=False)
                nc.tensor.matmul(po[:], lhsT=wl2b[:],
                                 rhs=mid_sb[1][:, c * CH:(c + 1) * CH],
                                 start=False, stop=True)
                ng = CH // N_LOCS
                nc.vector.reduce_sum(
                    lpT[:, c * ng:(c + 1) * ng],
                    po[:].rearrange("p (g l) -> p g l", l=N_LOCS),
                    axis=mybir.AxisListType.X)
            lpT2 = mlppool.tile([128, N_GRAPHS], f32)
            nc.vector.tensor_scalar(lpT2[:], lpT[:], 1.0 / N_LOCS, bl2c[:],
                                    mybir.AluOpType.mult, mybir.AluOpType.add)
            plp = ps.tile([N_GRAPHS, 128], f32, space="PSUM", tag="pscr")
            nc.tensor.matmul(plp[:], lhsT=lpT2[:], rhs=ident[:],
                             start=True, stop=True)
            out_sb = mlppool.tile([N_GRAPHS, 2 * GH], f32)
            nc.vector.tensor_copy(out_sb[:, GH:], plp[:])

            # ---------------- GAT layers ----------------
            psum_S = pspool.tile([N_GRAPHS, GH], f32, space="PSUM")

            for l in range(3):
                F, H = LCFG[l]
                RW = F + H                       # rhs/scatter width
                tab = T[l]
                for nt in range(NT):
                    idxs = sb.tile([128, K], i32, tag="idxs")
                    nc.sync.dma_start(idxs[:], inp["idx"][nt, :, :])
                    rels = sb.tile([128, K], f32, tag="rels")
                    nc.sync.dma_start(rels[:], inp["rel"][nt, :, :])
                    acc = psacc.tile([128, RW], f32, space="PSUM", tag="acc")
                    Kt = K_nt[nt]
                    for t in range(Kt):
                        g = sb.tile([128, TW], f32, tag="g")
                        nc.gpsimd.indirect_dma_start(
                            out=g[:], out_offset=None, in_=tab[:, :],
                            in_offset=bass.IndirectOffsetOnAxis(
                                ap=idxs[:, t:t + 1], axis=0))
                        pedge = sb.tile([128, 128], f32, tag="pedge")
                        nc.vector.tensor_tensor(
                            out=pedge[:], in0=rels[:, t:t + 1].to_broadcast([128, 128]),
                            in1=iota_row[:], op=mybir.AluOpType.is_equal)
                        pdst = sb.tile([128, 128], mybir.dt.float8e4, tag="pdst")
                        nc.sync.dma_start(pdst[:], inp["pdst"][nt, t, :, :])
                        pede = ps.tile([128, 3], f32, space="PSUM", tag="pscr")
                        nc.tensor.matmul(pede[:, :H], lhsT=pdst[:],
                                         rhs=ed_all[l][:, nt, :H],
                                         start=True, stop=True)
                        e = sb.tile([128, 3], f32, tag="e")
                        nc.vector.tensor_add(e[:, :H], g[:, F:F + H], pede[:, :H])
                        lk = sb.tile([128, 3], f32, tag="lk")
                        nc.scalar.activation(lk[:, :H], e[:, :H], AF.Prelu, alpha=NEG)
                        w = sb.tile([128, 3], f32, tag="w")
                        nc.scalar.activation(w[:, :H], lk[:, :H], AF.Exp)
                        rhs_t = sb.tile([128, RW], f32, tag="rhs_t")
                        oph = F // H
                        for hh in range(H):
                            nc.vector.tensor_scalar_mul(
                                rhs_t[:, hh * oph:(hh + 1) * oph],
                                g[:, hh * oph:(hh + 1) * oph], w[:, hh:hh + 1])
                        nc.vector.tensor_copy(rhs_t[:, F:F + H], w[:, :H])
                        nc.tensor.matmul(acc[:], lhsT=pedge[:], rhs=rhs_t[:],
                                         start=(t == 0), stop=(t == Kt - 1))
                    # epilogue for this node-tile
                    zc = sb.tile([128, 3], f32, tag="zc")
                    nc.vector.tensor_scalar_max(zc[:, :H], acc[:, F:F + H], 1e-30)
                    zr = sb.tile([128, 3], f32, tag="zr")
                    nc.vector.reciprocal(zr[:, :H], zc[:, :H])
                    u = sb.tile([128, F], f32, tag="u")
                    oph = F // H
                    for hh in range(H):
                        nc.vector.tensor_scalar_mul(
                            u[:, hh * oph:(hh + 1) * oph],
                            acc[:, hh * oph:(hh + 1) * oph], zr[:, hh:hh + 1])
                    ob = sb.tile([128, F], f32, tag="ob")
                    nc.vector.tensor_add(ob[:], u[:], bb_sb[l][:, :F])
                    if l < 2:
                        hin = sb.tile([128, F], f32, tag="hin")
                        nc.scalar.activation(hin[:], ob[:], AF.Prelu, alpha=NEG)
                        # transpose hin -> [F, 128] in two pieces
                        ph1 = ps.tile([128, 128], f32, space="PSUM", tag="pscr")
                        nc.tensor.matmul(ph1[:], lhsT=hin[:, 0:128], rhs=ident[:],
                                         start=True, stop=True)
                        hTa = sb.tile([128, 128], f32, tag="hTa")
                        nc.vector.tensor_copy(hTa[:], ph1[:])
                        ph2 = ps.tile([1, 128], f32, space="PSUM", tag="pscr")
                        nc.tensor.matmul(ph2[:], lhsT=hin[:, 128:129], rhs=ident[:],
                                         start=True, stop=True)
                        hTb = sb.tile([1, 128], f32, tag="hTb")
                        nc.vector.tensor_copy(hTb[:], ph2[:])
                        nF2, nH2 = LCFG[l + 1]
                        ptab = ps.tile([128, nF2 + 2 * nH2], f32, space="PSUM",
                                       tag="pscr")
                        nc.tensor.matmul(ptab[:], lhsT=hTa[:],
                                         rhs=rhs_a[l + 1][:],
                                         start=True, stop=False)
                        nc.tensor.matmul(ptab[:], lhsT=hTb[:],
                                         rhs=rhs_b[l + 1][:],
                                         start=False, stop=True)
                        stg = sb.tile([128, TW], f32, tag="stg")
                        nc.vector.memset(stg[:, nF2 + 2 * nH2:], 0.0)
                        nc.vector.tensor_copy(stg[:, :nF2 + 2 * nH2], ptab[:])
                        nc.sync.dma_start(
                            Tsh[l][nt * 128:(nt + 1) * 128, :], stg[:])
                        nc.vector.tensor_copy(
                            ed_all[l + 1][:, nt, :nH2],
                            ptab[:, nF2 + nH2:nF2 + 2 * nH2])
                    else:
                        bpt = sb.tile([128, N_GRAPHS], f32, tag="bpt")
                        nc.sync.dma_start(bpt[:], inp["bpool"][nt, :, :])
                        nc.tensor.matmul(psum_S[:], lhsT=bpt[:], rhs=ob[:],
                                         start=(nt == 0), stop=(nt == NT - 1))
                if l < 2:
                    # zero row of next table, then replicate shards
                    zz2 = sb.tile([1, TW], f32, tag="zrow")
                    nc.vector.memset(zz2[:], 0.0)
                    nF2, nH2 = LCFG[l + 1]
                    nc.vector.memset(zz2[:, nF2:nF2 + nH2], -1000.0)
                    nc.sync.dma_start(T[l + 1][ZROW:ZROW + 1, :], zz2[:])
                    nc.gpsimd.collective_compute(
                        "AllGather", mybir.AluOpType.bypass, replica_groups=RG,
                        ins=[Tsh[l][:, :]], outs=[T[l + 1][0:NPAD, :]])

            # pooling: AllReduce of per-shard sums, then divide by counts
            ssb = sb.tile([N_GRAPHS, GH], f32, tag="ssb")
            nc.vector.tensor_copy(ssb[:], psum_S[:])
            nc.sync.dma_start(s_in[:, :], ssb[:])
            nc.gpsimd.collective_compute(
                "AllReduce", mybir.AluOpType.add, replica_groups=RG,
                ins=[s_in[:, :]], outs=[s_out[:, :]])
            sfull = sb.tile([N_GRAPHS, GH], f32, tag="sfull")
            nc.sync.dma_start(sfull[:], s_out[:, :])
            civ = sb.tile([N_GRAPHS, 1], f32, tag="civ")
            nc.sync.dma_start(civ[:], inp["cntinv"][:, :])
            nc.vector.tensor_scalar_mul(out_sb[:, 0:GH], sfull[:], civ[:])
            nc.sync.dma_start(out[:, :], out_sb[:])

    return nc


def kernel(**inputs):
    key = "k"
    in_maps, Kinfo = _host_prep(**inputs)
    if key not in _CACHE or _CACHE[key][1] != Kinfo:
        nc = _build(Kinfo)
        _CACHE[key] = (nc, Kinfo)
    nc = _CACHE[key][0]
    from concourse.bass_utils import run_bass_kernel_spmd
    res = run_bass_kernel_spmd(nc, in_maps, core_ids=list(range(N_CORES)))
    return np.asarray(res.results[0]["out"])


# revision 9
# speedup vs baseline: 1.1257x; 1.0005x over previous
"""GAT backbone (3-layer GATConv + graph pooling + loc-MLP) on 8 Trainium2
NeuronCores.

Strategy: dst-sharded edges. Each core owns a contiguous range of 6272
destination nodes (49 node-tiles of 128). Edges (with self-loops) are sorted
by dst on the host and padded so every (core, node-tile) has exactly K
128-edge tiles. Per edge-tile the core gathers table rows [h'|es|ed] for the
edge sources via indirect DMA, builds the dst one-hot on-device (iota compare
+ PE transpose), computes softmax weights w = exp(leaky(es_src + ed_dst)),
and scatter-accumulates [w*h' | w] into PSUM with a one-hot matmul. The
normalized output is transformed (h_in @ [W|As_eff|Ad_eff]) into the next
layer's table, which is replicated across cores with an AllGather. Graph mean
pool is a per-shard matmul with a host-built one-hot, AllReduce-summed across
cores. The loc-MLP is computed redundantly on every core.
"""
import numpy as np

# ---------------------------------------------------------------------------
# runtime patch: this walrus build accepts at most ONE sync-wait command per
# instruction; Tile attaches several. Split extras into single-wait NOPs.
# ---------------------------------------------------------------------------
_PATCHED = [False]


def _install_tile_patch():
    if _PATCHED[0]:
        return
    import concourse.mybir as mybir
    from concourse.tile import TileContext
    from concourse.vector_clock import ScopedClock

    ctr = [0]

    def _split(insts):
        new = []
        for inst in insts:
            si = getattr(inst, "sync_info", None)
            try:
                ow = si.on_wait if si is not None else None
            except Exception:
                ow = None
            if ow is not None and len(ow) > 1:
                waits = list(ow)
                for w in waits[:-1]:
                    ctr[0] += 1
                    nop = mybir.InstNoOp(name=f"wsplit-{ctr[0]}", ins=[], outs=[])
                    nop.engine = inst.engine
                    nop.sync_info = mybir.SyncInfo(on_wait=[w], on_update=[])
                    new.append(nop)
                si.on_wait = waits[-1:]
            new.append(inst)
        insts[:] = new

    orig_lower = TileContext._lower_ordered_insts

    def patched_lower(self, ordered):
        for insts in ordered.values():
            _split(insts)
        return orig_lower(self, ordered)

    def patched_drain(self, tick_clock, wait_clock):
        drain_inst = self.nc.sync.drain()
        wait_clock.add_sem_waits(
            drain_inst.ins, ScopedClock({None: tick_clock.global_clock})
        )
        si = drain_inst.ins.sync_info
        if si is not None and si.on_wait and len(si.on_wait) > 1:
            waits = list(si.on_wait)
            si.on_wait = waits[:1]
            for w in waits[1:]:
                extra = self.nc.sync.drain()
                esi = extra.ins.sync_info
                if esi is None:
                    extra.ins.sync_info = mybir.SyncInfo(on_wait=[w], on_update=[])
                else:
                    esi.on_wait = [w]
        self.nc.all_engine_barrier()
        assert self.sems is not None
        popped = self.nc._tile_sem_poison_stack.pop()
        assert popped is self._sem_poison
        self.nc.clear_and_free_semaphores(list(self.sems.allocated().values()))
        self.nc.all_engine_barrier()

    TileContext._lower_ordered_insts = patched_lower
    TileContext._drain_and_barrier = patched_drain
    _PATCHED[0] = True


# ---------------------------------------------------------------------------
# problem constants (hardcoded per contract)
# ---------------------------------------------------------------------------
N_NODES = 50000
N_EDGES = 800000
N_GRAPHS = 64
N_LOCS = 50
HEADS = 3
OPH = 43                    # out per head
MID = HEADS * OPH           # 129
GH = 128                    # gat hidden (layer 2 out)
MLPH = 256
NEG = 0.2
N_CORES = 8
SHARD = 6272                # 49 * 128 dst nodes per core
NT = SHARD // 128           # 49 node-tiles
NPAD = SHARD * N_CORES      # 50176
ZROW = NPAD                 # zero row index
TW = 136                    # table row width (f32): h'(<=129)|es|ed|pad

_CACHE = {}


def _host_prep(x, loc, edge_index, batch, W0, as0, ad0, b0, W1, as1, ad1, b1,
               W2, as2, ad2, b2, Wl1, bl1, Wl2, bl2):
    f32 = np.float32
    src = np.concatenate([edge_index[0], np.arange(N_NODES, dtype=np.int64)])
    dst = np.concatenate([edge_index[1], np.arange(N_NODES, dtype=np.int64)])

    # group edges per (core, node-tile)
    order = np.argsort(dst, kind="stable")
    src = src[order].astype(np.int64)
    dst = dst[order].astype(np.int64)
    tile_of = dst // 128                     # global node-tile id, 0..391
    # counts per global tile (node-tiles beyond 50000 have 0)
    n_tiles_total = NPAD // 128              # 392
    counts = np.bincount(tile_of, minlength=n_tiles_total)
    K = int(np.ceil(counts.max() / 128))
    cpt = counts.reshape(N_CORES, NT)
    K_nt = tuple(int(np.ceil(cpt[:, t].max() / 128)) for t in range(NT))
    starts = np.zeros(n_tiles_total + 1, np.int64)
    np.cumsum(counts, out=starts[1:])

    idx_all = np.full((N_CORES, NT, K * 128), ZROW, np.int32)
    rel_all = np.zeros((N_CORES, NT, K * 128), np.float32)
    for g in range(n_tiles_total):
        c, t = divmod(g, NT)
        s, e = starts[g], starts[g + 1]
        cnt = e - s
        idx_all[c, t, :cnt] = src[s:e]
        rel_all[c, t, :cnt] = (dst[s:e] - g * 128).astype(np.float32)
    # device layout [NT, 128, K]: edge j of tile t at [t, j%128, j//128]
    idx_dev = idx_all.reshape(N_CORES, NT, K, 128).transpose(0, 1, 3, 2).copy()
    rel_dev = rel_all.reshape(N_CORES, NT, K, 128).transpose(0, 1, 3, 2).copy()
    # host-built P_dst one-hot per edge-tile, fp8: [NT, K, 128 dst, 128 edges]
    import ml_dtypes
    rel_i = rel_all.reshape(N_CORES, NT, K, 128).astype(np.int64)
    eye8 = np.eye(128, dtype=ml_dtypes.float8_e4m3)
    pdst_dev = np.ascontiguousarray(np.moveaxis(eye8[:, rel_i], 0, -2))

    # pooling one-hot per core [NT, 128, 64] and counts
    bp = np.zeros((N_CORES, NT, 128, N_GRAPHS), f32)
    node = np.arange(NPAD)
    valid = node < N_NODES
    gid = np.where(valid, batch[np.minimum(node, N_NODES - 1)], 0)
    onehot = np.zeros((NPAD, N_GRAPHS), f32)
    onehot[valid, gid[valid]] = 1.0
    bp[:] = onehot.reshape(N_CORES, NT, 128, N_GRAPHS)
    cnt = np.bincount(batch, minlength=N_GRAPHS).astype(f32)
    cntinv = (1.0 / np.maximum(cnt, 1.0)).reshape(N_GRAPHS, 1).astype(f32)

    # weight packing: rhs_l = [W_l | W_l@Amat_s | W_l@Amat_d]
    def amat(a):
        h, o = a.shape
        m = np.zeros((h * o, h), f32)
        for i in range(h):
            m[i * o:(i + 1) * o, i] = a[i]
        return m

    rhs0 = np.concatenate([W0, W0 @ amat(as0), W0 @ amat(ad0)], axis=1).astype(f32)
    rhs1 = np.concatenate([W1, W1 @ amat(as1), W1 @ amat(ad1)], axis=1).astype(f32)
    rhs2 = np.concatenate([W2, W2 @ amat(as2), W2 @ amat(ad2)], axis=1).astype(f32)

    xT = np.zeros((6, NPAD), f32)
    xT[:, :N_NODES] = np.asarray(x, f32).T
    xT_own = xT.reshape(6, N_CORES, SHARD).transpose(1, 0, 2).copy()

    bb0 = np.tile(np.asarray(b0, f32)[None, :], (128, 1))
    bb1 = np.tile(np.asarray(b1, f32)[None, :], (128, 1))
    bb2 = np.tile(np.asarray(b2, f32)[None, :], (128, 1))

    locT = np.asarray(loc, f32).reshape(N_GRAPHS * N_LOCS, 2).T.copy()
    ident = np.eye(128, dtype=f32)

    common = dict(
        xT=xT, rhs0=rhs0, rhs1=rhs1, rhs2=rhs2,
        bb0=bb0, bb1=bb1, bb2=bb2, cntinv=cntinv,
        locT=locT, wl1=np.asarray(Wl1, f32), bl1c=np.asarray(bl1, f32).reshape(-1, 1),
        wl2=np.asarray(Wl2, f32), bl2c=np.asarray(bl2, f32).reshape(-1, 1),
        ident=ident,
    )
    in_maps = []
    for c in range(N_CORES):
        m = dict(common)
        m["idx"] = idx_dev[c]
        m["rel"] = rel_dev[c]
        m["bpool"] = bp[c]
        m["xTo"] = xT_own[c]
        m["pdst"] = pdst_dev[c]
        in_maps.append(m)
    return in_maps, (K, K_nt)


def _build(Kinfo):
    K, K_nt = Kinfo
    _install_tile_patch()
    import concourse.bass as bass
    import concourse.mybir as mybir
    import concourse.tile as tile

    f32 = mybir.dt.float32
    i32 = mybir.dt.int32
    AF = mybir.ActivationFunctionType

    nc = bass.Bass(num_devices=N_CORES)

    inp = {}
    for name, shape, dt in [
        ("xT", [6, NPAD], f32), ("xTo", [6, SHARD], f32),
        ("idx", [NT, 128, K], i32), ("rel", [NT, 128, K], f32),
        ("pdst", [NT, K, 128, 128], mybir.dt.float8e4),
        ("bpool", [NT, 128, N_GRAPHS], f32), ("cntinv", [N_GRAPHS, 1], f32),
        ("rhs0", [6, MID + 6], f32), ("rhs1", [MID, MID + 6], f32),
        ("rhs2", [MID, GH + 2], f32),
        ("bb0", [128, MID], f32), ("bb1", [128, MID], f32), ("bb2", [128, GH], f32),
        ("locT", [2, N_GRAPHS * N_LOCS], f32), ("wl1", [2, MLPH], f32),
        ("bl1c", [MLPH, 1], f32), ("wl2", [MLPH, GH], f32), ("bl2c", [GH, 1], f32),
        ("ident", [128, 128], f32),
    ]:
        inp[name] = nc.dram_tensor(name, shape, dt, kind="ExternalInput")

    out = nc.dram_tensor("out", [N_GRAPHS, GH * 2], f32, kind="ExternalOutput")

    # tables
    T = [
        nc.dram_tensor("T0", [NPAD + 1, TW], f32, kind="Internal"),
        nc.dram_tensor("T1", [NPAD + 1, TW], f32, kind="Internal",
                       addr_space="Shared"),
        nc.dram_tensor("T2", [NPAD + 1, TW], f32, kind="Internal",
                       addr_space="Shared"),
    ]
    Tsh = [
        nc.dram_tensor("Tsh1", [SHARD, TW], f32, kind="Internal"),
        nc.dram_tensor("Tsh2", [SHARD, TW], f32, kind="Internal"),
    ]
    s_in = nc.dram_tensor("s_in", [N_GRAPHS, GH], f32, kind="Internal")
    s_out = nc.dram_tensor("s_out", [N_GRAPHS, GH], f32, kind="Internal",
                           addr_space="Shared")

    # per-layer config: (feat width F, heads H) — rhs width = F + H
    LCFG = [(MID, HEADS), (MID, HEADS), (GH, 1)]
    RG = [list(range(N_CORES))]

    with tile.TileContext(nc) as tc:
        with tc.tile_pool(name="const", bufs=1) as cp, \
             tc.tile_pool(name="sb", bufs=4) as sb, \
             tc.tile_pool(name="ed", bufs=1) as edp, \
             tc.tile_pool(name="mlp", bufs=1) as mlppool, \
             tc.tile_pool(name="ps", bufs=4, space="PSUM") as ps, \
             tc.tile_pool(name="psacc", bufs=2, space="PSUM") as psacc, \
             tc.tile_pool(name="pspool", bufs=1, space="PSUM") as pspool:

            ident = cp.tile([128, 128], f32)
            nc.sync.dma_start(ident[:], inp["ident"][:, :])
            iota_row = cp.tile([128, 128], f32)
            nc.gpsimd.iota(iota_row[:], pattern=[[1, 128]], base=0,
                        channel_multiplier=0,
                        allow_small_or_imprecise_dtypes=True)

            rhs_sb = []      # layer 0: [6, 135]
            t0r = cp.tile([6, MID + 6], f32, tag="rhs0")
            nc.sync.dma_start(t0r[:], inp["rhs0"][:, :])
            rhs_sb.append(t0r)
            rhs_a, rhs_b = {}, {}   # layers 1,2: split [128,W] + [1,W]
            for l, nm in [(1, "rhs1"), (2, "rhs2")]:
                w = inp[nm].shape[1]
                ta = cp.tile([128, w], f32, tag=f"rhsa{l}")
                nc.sync.dma_start(ta[:], inp[nm][0:128, :])
                tb = cp.tile([1, w], f32, tag=f"rhsb{l}")
                nc.sync.dma_start(tb[:], inp[nm][128:129, :])
                rhs_a[l], rhs_b[l] = ta, tb
            bb_sb = []
            for l, nm in enumerate(["bb0", "bb1", "bb2"]):
                t = cp.tile(list(inp[nm].shape), f32, tag=f"bb{l}")
                nc.sync.dma_start(t[:], inp[nm][:, :])
                bb_sb.append(t)

            # ed values for own shard, per layer: [128, NT, H]
            ed_all = [edp.tile([128, NT, 3], mybir.dt.bfloat16, tag=f"edall{l}", name=f"edall{l}") for l in range(3)]

            # ---------------- stage A: build T0 for all nodes ----------------
            for j in range(NPAD // 128):
                xs = sb.tile([6, 128], f32, tag="xs")
                nc.sync.dma_start(xs[:], inp["xT"][:, j * 128:(j + 1) * 128])
                ptab = ps.tile([128, MID + 6], f32, space="PSUM", tag="pscr")
                nc.tensor.matmul(ptab[:], lhsT=xs[:], rhs=rhs_sb[0][:],
                                 start=True, stop=True)
                stg = sb.tile([128, TW], f32, tag="stg0")
                nc.vector.memset(stg[:, MID + 6:], 0.0)
                nc.vector.tensor_copy(stg[:, :MID + 6], ptab[:])
                nc.sync.dma_start(T[0][j * 128:(j + 1) * 128, :], stg[:])
            zz = sb.tile([1, TW], f32, tag="zrow")
            nc.vector.memset(zz[:], 0.0)
            nc.vector.memset(zz[:, MID:MID + 3], -1000.0)
            nc.sync.dma_start(T[0][ZROW:ZROW + 1, :], zz[:])
            # ed0 for own shard (from per-core xTo input)
            for t in range(NT):
                xo = sb.tile([6, 128], f32, tag="xs")
                nc.sync.dma_start(xo[:], inp["xTo"][:, t * 128:(t + 1) * 128])
                pe0 = ps.tile([128, 6], f32, space="PSUM", tag="pscr")
                nc.tensor.matmul(pe0[:], lhsT=xo[:], rhs=rhs_sb[0][:, MID:MID + 6],
                                 start=True, stop=True)
                nc.vector.tensor_copy(ed_all[0][:, t, :], pe0[:, 3:6])

            # ---------------- loc MLP (independent) ----------------
            locT = mlppool.tile([2, N_GRAPHS * N_LOCS], f32)
            nc.sync.dma_start(locT[:], inp["locT"][:, :])
            wl1 = mlppool.tile([2, MLPH], f32)
            nc.sync.dma_start(wl1[:], inp["wl1"][:, :])
            bl1c = mlppool.tile([128, 2], f32)
            nc.sync.dma_start(bl1c[:], inp["bl1c"][:, 0:1].rearrange("(h p) o -> p (h o)", p=128))
            wl2a = mlppool.tile([128, GH], f32)
            nc.sync.dma_start(wl2a[:], inp["wl2"][0:128, :])
            wl2b = mlppool.tile([128, GH], f32)
            nc.sync.dma_start(wl2b[:], inp["wl2"][128:256, :])
            bl2c = mlppool.tile([GH, 1], f32)
            nc.sync.dma_start(bl2c[:], inp["bl2c"][:, :])
            mid_sb = [mlppool.tile([128, N_GRAPHS * N_LOCS], f32, tag=f"mid{h}", name=f"mid{h}")
                      for h in range(2)]
            CH = 400
            nch = (N_GRAPHS * N_LOCS) // CH
            for h in range(2):
                for c in range(nch):
                    pm = ps.tile([128, CH], f32, space="PSUM", tag="pscr")
                    nc.tensor.matmul(pm[:], lhsT=wl1[:, h * 128:(h + 1) * 128],
                                     rhs=locT[:, c * CH:(c + 1) * CH],
                                     start=True, stop=True)
                    nc.scalar.activation(mid_sb[h][:, c * CH:(c + 1) * CH], pm[:],
                                         AF.Tanh, bias=bl1c[:, h:h + 1])
            lpT = mlppool.tile([128, N_GRAPHS], f32)
            for c in range(nch):
                po = ps.tile([128, CH], f32, space="PSUM", tag="pscr")
                nc.tensor.matmul(po[:], lhsT=wl2a[:],
                                 rhs=mid_sb[0][:, c * CH:(c + 1) * CH],
                                 start=True, stop=False)
                nc.tensor.matmul(po[:], lhsT=wl2b[:],
                                 rhs=mid_sb[1][:, c * CH:(c + 1) * CH],
                                 start=False, stop=True)
                ng = CH // N_LOCS
                nc.vector.reduce_sum(
                    lpT[:, c * ng:(c + 1) * ng],
                    po[:].rearrange("p (g l) -> p g l", l=N_LOCS),
                    axis=mybir.AxisListType.X)
            lpT2 = mlppool.tile([128, N_GRAPHS], f32)
            nc.vector.tensor_scalar(lpT2[:], lpT[:], 1.0 / N_LOCS, bl2c[:],
                                    mybir.AluOpType.mult, mybir.AluOpType.add)
            plp = ps.tile([N_GRAPHS, 128], f32, space="PSUM", tag="pscr")
            nc.tensor.matmul(plp[:], lhsT=lpT2[:], rhs=ident[:],
                             start=True, stop=True)
            out_sb = mlppool.tile([N_GRAPHS, 2 * GH], f32)
            nc.vector.tensor_copy(out_sb[:, GH:], plp[:])

            # ---------------- GAT layers ----------------
            psum_S = pspool.tile([N_GRAPHS, GH], f32, space="PSUM")

            for l in range(3):
                F, H = LCFG[l]
                RW = F + H                       # rhs/scatter width
                tab = T[l]
                for nt in range(NT):
                    idxs = sb.tile([128, K], i32, tag="idxs")
                    nc.sync.dma_start(idxs[:], inp["idx"][nt, :, :])
                    rels = sb.tile([128, K], f32, tag="rels")
                    nc.sync.dma_start(rels[:], inp["rel"][nt, :, :])
                    acc = psacc.tile([128, RW], f32, space="PSUM", tag="acc")
                    Kt = K_nt[nt]
                    for t in range(Kt):
                        g = sb.tile([128, TW], f32, tag="g")
                        nc.gpsimd.indirect_dma_start(
                            out=g[:], out_offset=None, in_=tab[:, :],
                            in_offset=bass.IndirectOffsetOnAxis(
                                ap=idxs[:, t:t + 1], axis=0))
                        pedge = sb.tile([128, 128], f32, tag="pedge")
                        nc.vector.tensor_tensor(
                            out=pedge[:], in0=rels[:, t:t + 1].to_broadcast([128, 128]),
                            in1=iota_row[:], op=mybir.AluOpType.is_equal)
                        pdst = sb.tile([128, 128], mybir.dt.float8e4, tag="pdst")
                        nc.sync.dma_start(pdst[:], inp["pdst"][nt, t, :, :])
                        pede = ps.tile([128, 3], f32, space="PSUM", tag="pscr")
                        nc.tensor.matmul(pede[:, :H], lhsT=pdst[:],
                                         rhs=ed_all[l][:, nt, :H],
                                         start=True, stop=True)
                        e = sb.tile([128, 3], f32, tag="e")
                        nc.vector.tensor_add(e[:, :H], g[:, F:F + H], pede[:, :H])
                        lk = sb.tile([128, 3], f32, tag="lk")
                        nc.scalar.activation(lk[:, :H], e[:, :H], AF.Prelu, alpha=NEG)
                        w = sb.tile([128, 3], f32, tag="w")
                        nc.scalar.activation(w[:, :H], lk[:, :H], AF.Exp)
                        rhs_t = sb.tile([128, RW], f32, tag="rhs_t")
                        oph = F // H
                        for hh in range(H):
                            nc.vector.tensor_scalar_mul(
                                rhs_t[:, hh * oph:(hh + 1) * oph],
                                g[:, hh * oph:(hh + 1) * oph], w[:, hh:hh + 1])
                        nc.vector.tensor_copy(rhs_t[:, F:F + H], w[:, :H])
                        nc.tensor.matmul(acc[:], lhsT=pedge[:], rhs=rhs_t[:],
                                         start=(t == 0), stop=(t == Kt - 1))
                    # epilogue for this node-tile
                    zc = sb.tile([128, 3], f32, tag="zc")
                    nc.vector.tensor_scalar_max(zc[:, :H], acc[:, F:F + H], 1e-30)
                    zr = sb.tile([128, 3], f32, tag="zr")
                    nc.vector.reciprocal(zr[:, :H], zc[:, :H])
                    u = sb.tile([128, F], f32, tag="u")
                    oph = F // H
                    for hh in range(H):
                        nc.vector.tensor_scalar_mul(
                            u[:, hh * oph:(hh + 1) * oph],
                            acc[:, hh * oph:(hh + 1) * oph], zr[:, hh:hh + 1])
                    ob = sb.tile([128, F], f32, tag="ob")
                    nc.vector.tensor_add(ob[:], u[:], bb_sb[l][:, :F])
                    if l < 2:
                        hin = sb.tile([128, F], f32, tag="hin")
                        nc.scalar.activation(hin[:], ob[:], AF.Prelu, alpha=NEG)
                        # transpose hin -> [F, 128] in two pieces
                        ph1 = ps.tile([128, 128], f32, space="PSUM", tag="pscr")
                        nc.tensor.matmul(ph1[:], lhsT=hin[:, 0:128], rhs=ident[:],
                                         start=True, stop=True)
                        hTa = sb.tile([128, 128], f32, tag="hTa")
                        nc.vector.tensor_copy(hTa[:], ph1[:])
                        ph2 = ps.tile([1, 128], f32, space="PSUM", tag="pscr")
                        nc.tensor.matmul(ph2[:], lhsT=hin[:, 128:129], rhs=ident[:],
                                         start=True, stop=True)
                        hTb = sb.tile([1, 128], f32, tag="hTb")
                        nc.vector.tensor_copy(hTb[:], ph2[:])
                        nF2, nH2 = LCFG[l + 1]
                        ptab = ps.tile([128, nF2 + 2 * nH2], f32, space="PSUM",
                                       tag="pscr")
                        nc.tensor.matmul(ptab[:], lhsT=hTa[:],
                                         rhs=rhs_a[l + 1][:],
                                         start=True, stop=False)
                        nc.tensor.matmul(ptab[:], lhsT=hTb[:],
                                         rhs=rhs_b[l + 1][:],
                                         start=False, stop=True)
                        stg = sb.tile([128, TW], f32, tag="stg")
                        nc.vector.memset(stg[:, nF2 + 2 * nH2:], 0.0)
                        nc.vector.tensor_copy(stg[:, :nF2 + 2 * nH2], ptab[:])
                        nc.sync.dma_start(
                            Tsh[l][nt * 128:(nt + 1) * 128, :], stg[:])
                        nc.vector.tensor_copy(
                            ed_all[l + 1][:, nt, :nH2],
                            ptab[:, nF2 + nH2:nF2 + 2 * nH2])
                    else:
                        bpt = sb.tile([128, N_GRAPHS], f32, tag="bpt")
                        nc.sync.dma_start(bpt[:], inp["bpool"][nt, :, :])
                        nc.tensor.matmul(psum_S[:], lhsT=bpt[:], rhs=ob[:],
                                         start=(nt == 0), stop=(nt == NT - 1))
                if l < 2:
                    # zero row of next table, then replicate shards
                    zz2 = sb.tile([1, TW], f32, tag="zrow")
                    nc.vector.memset(zz2[:], 0.0)
                    nF2, nH2 = LCFG[l + 1]
                    nc.vector.memset(zz2[:, nF2:nF2 + nH2], -1000.0)
                    nc.sync.dma_start(T[l + 1][ZROW:ZROW + 1, :], zz2[:])
                    nc.gpsimd.collective_compute(
                        "AllGather", mybir.AluOpType.bypass, replica_groups=RG,
                        ins=[Tsh[l][:, :]], outs=[T[l + 1][0:NPAD, :]])

            # pooling: AllReduce of per-shard sums, then divide by counts
            ssb = sb.tile([N_GRAPHS, GH], f32, tag="ssb")
            nc.vector.tensor_copy(ssb[:], psum_S[:])
            nc.sync.dma_start(s_in[:, :], ssb[:])
            nc.gpsimd.collective_compute(
                "AllReduce", mybir.AluOpType.add, replica_groups=RG,
                ins=[s_in[:, :]], outs=[s_out[:, :]])
            sfull = sb.tile([N_GRAPHS, GH], f32, tag="sfull")
            nc.sync.dma_start(sfull[:], s_out[:, :])
            civ = sb.tile([N_GRAPHS, 1], f32, tag="civ")
            nc.sync.dma_start(civ[:], inp["cntinv"][:, :])
            nc.vector.tensor_scalar_mul(out_sb[:, 0:GH], sfull[:], civ[:])
            nc.sync.dma_start(out[:, :], out_sb[:])

    return nc


def kernel(**inputs):
    key = "k"
    in_maps, Kinfo = _host_prep(**inputs)
    if key not in _CACHE or _CACHE[key][1] != Kinfo:
        nc = _build(Kinfo)
        _CACHE[key] = (nc, Kinfo)
    nc = _CACHE[key][0]
    from concourse.bass_utils import run_bass_kernel_spmd
    res = run_bass_kernel_spmd(nc, in_maps, core_ids=list(range(N_CORES)))
    return np.asarray(res.results[0]["out"])


# revision 10
# speedup vs baseline: 1.3820x; 1.2277x over previous
"""GAT backbone (3-layer GATConv + graph pooling + loc-MLP) on 8 Trainium2
NeuronCores.

Strategy: dst-sharded edges. Each core owns a contiguous range of 6272
destination nodes (49 node-tiles of 128). Edges (with self-loops) are sorted
by dst on the host and padded so every (core, node-tile) has exactly K
128-edge tiles. Per edge-tile the core gathers table rows [h'|es|ed] for the
edge sources via indirect DMA, builds the dst one-hot on-device (iota compare
+ PE transpose), computes softmax weights w = exp(leaky(es_src + ed_dst)),
and scatter-accumulates [w*h' | w] into PSUM with a one-hot matmul. The
normalized output is transformed (h_in @ [W|As_eff|Ad_eff]) into the next
layer's table, which is replicated across cores with an AllGather. Graph mean
pool is a per-shard matmul with a host-built one-hot, AllReduce-summed across
cores. The loc-MLP is computed redundantly on every core.
"""
import numpy as np

# ---------------------------------------------------------------------------
# runtime patch: this walrus build accepts at most ONE sync-wait command per
# instruction; Tile attaches several. Split extras into single-wait NOPs.
# ---------------------------------------------------------------------------
_PATCHED = [False]


def _install_tile_patch():
    if _PATCHED[0]:
        return
    import concourse.mybir as mybir
    from concourse.tile import TileContext
    from concourse.vector_clock import ScopedClock

    ctr = [0]

    def _split(insts):
        new = []
        for inst in insts:
            si = getattr(inst, "sync_info", None)
            try:
                ow = si.on_wait if si is not None else None
            except Exception:
                ow = None
            if ow is not None and len(ow) > 1:
                waits = list(ow)
                for w in waits[:-1]:
                    ctr[0] += 1
                    nop = mybir.InstNoOp(name=f"wsplit-{ctr[0]}", ins=[], outs=[])
                    nop.engine = inst.engine
                    nop.sync_info = mybir.SyncInfo(on_wait=[w], on_update=[])
                    new.append(nop)
                si.on_wait = waits[-1:]
            new.append(inst)
        insts[:] = new

    orig_lower = TileContext._lower_ordered_insts

    def patched_lower(self, ordered):
        for insts in ordered.values():
            _split(insts)
        return orig_lower(self, ordered)

    def patched_drain(self, tick_clock, wait_clock):
        drain_inst = self.nc.sync.drain()
        wait_clock.add_sem_waits(
            drain_inst.ins, ScopedClock({None: tick_clock.global_clock})
        )
        si = drain_inst.ins.sync_info
        if si is not None and si.on_wait and len(si.on_wait) > 1:
            waits = list(si.on_wait)
            si.on_wait = waits[:1]
            for w in waits[1:]:
                extra = self.nc.sync.drain()
                esi = extra.ins.sync_info
                if esi is None:
                    extra.ins.sync_info = mybir.SyncInfo(on_wait=[w], on_update=[])
                else:
                    esi.on_wait = [w]
        self.nc.all_engine_barrier()
        assert self.sems is not None
        popped = self.nc._tile_sem_poison_stack.pop()
        assert popped is self._sem_poison
        self.nc.clear_and_free_semaphores(list(self.sems.allocated().values()))
        self.nc.all_engine_barrier()

    TileContext._lower_ordered_insts = patched_lower
    TileContext._drain_and_barrier = patched_drain
    _PATCHED[0] = True


# ---------------------------------------------------------------------------
# problem constants (hardcoded per contract)
# ---------------------------------------------------------------------------
N_NODES = 50000
N_EDGES = 800000
N_GRAPHS = 64
N_LOCS = 50
HEADS = 3
OPH = 43                    # out per head
MID = HEADS * OPH           # 129
GH = 128                    # gat hidden (layer 2 out)
MLPH = 256
NEG = 0.2
N_CORES = 8
SHARD = 6272                # 49 * 128 dst nodes per core
NT = SHARD // 128           # 49 node-tiles
NPAD = SHARD * N_CORES      # 50176
ZROW = NPAD                 # zero row index
TW = 136                    # table row width (f32): h'(<=129)|es|ed|pad

_CACHE = {}


def _host_prep(x, loc, edge_index, batch, W0, as0, ad0, b0, W1, as1, ad1, b1,
               W2, as2, ad2, b2, Wl1, bl1, Wl2, bl2):
    f32 = np.float32
    src = np.concatenate([edge_index[0], np.arange(N_NODES, dtype=np.int64)])
    dst = np.concatenate([edge_index[1], np.arange(N_NODES, dtype=np.int64)])

    # group edges per (core, node-tile)
    order = np.argsort(dst, kind="stable")
    src = src[order].astype(np.int64)
    dst = dst[order].astype(np.int64)
    tile_of = dst // 128                     # global node-tile id, 0..391
    # counts per global tile (node-tiles beyond 50000 have 0)
    n_tiles_total = NPAD // 128              # 392
    counts = np.bincount(tile_of, minlength=n_tiles_total)
    K = int(np.ceil(counts.max() / 128))
    cpt = counts.reshape(N_CORES, NT)
    K_nt = tuple(int(np.ceil(cpt[:, t].max() / 128)) for t in range(NT))
    starts = np.zeros(n_tiles_total + 1, np.int64)
    np.cumsum(counts, out=starts[1:])

    idx_all = np.full((N_CORES, NT, K * 128), ZROW, np.int32)
    rel_all = np.zeros((N_CORES, NT, K * 128), np.float32)
    for g in range(n_tiles_total):
        c, t = divmod(g, NT)
        s, e = starts[g], starts[g + 1]
        cnt = e - s
        idx_all[c, t, :cnt] = src[s:e]
        rel_all[c, t, :cnt] = (dst[s:e] - g * 128).astype(np.float32)
    # device layout [NT, 128, K]: edge j of tile t at [t, j%128, j//128]
    idx_dev = idx_all.reshape(N_CORES, NT, K, 128).transpose(0, 1, 3, 2).copy()
    rel_dev = rel_all.reshape(N_CORES, NT, K, 128).transpose(0, 1, 3, 2).copy()
    # host-built P_dst one-hot per edge-tile, fp8: [NT, K, 128 dst, 128 edges]
    import ml_dtypes
    rel_i = rel_all.reshape(N_CORES, NT, K, 128).astype(np.int64)
    eye8 = np.eye(128, dtype=ml_dtypes.float8_e4m3)
    pdst_dev = np.ascontiguousarray(np.moveaxis(eye8[:, rel_i], 0, -2))

    # pooling one-hot per core [NT, 128, 64] and counts
    bp = np.zeros((N_CORES, NT, 128, N_GRAPHS), f32)
    node = np.arange(NPAD)
    valid = node < N_NODES
    gid = np.where(valid, batch[np.minimum(node, N_NODES - 1)], 0)
    onehot = np.zeros((NPAD, N_GRAPHS), f32)
    onehot[valid, gid[valid]] = 1.0
    bp[:] = onehot.reshape(N_CORES, NT, 128, N_GRAPHS)
    cnt = np.bincount(batch, minlength=N_GRAPHS).astype(f32)
    cntinv = (1.0 / np.maximum(cnt, 1.0)).reshape(N_GRAPHS, 1).astype(f32)

    # weight packing: rhs_l = [W_l | W_l@Amat_s | W_l@Amat_d]
    def amat(a):
        h, o = a.shape
        m = np.zeros((h * o, h), f32)
        for i in range(h):
            m[i * o:(i + 1) * o, i] = a[i]
        return m

    rhs0 = np.concatenate([W0, W0 @ amat(as0), W0 @ amat(ad0)], axis=1).astype(f32)
    rhs1 = np.concatenate([W1, W1 @ amat(as1), W1 @ amat(ad1)], axis=1).astype(f32)
    rhs2 = np.concatenate([W2, W2 @ amat(as2), W2 @ amat(ad2)], axis=1).astype(f32)

    xT = np.zeros((6, NPAD), f32)
    xT[:, :N_NODES] = np.asarray(x, f32).T
    xT_own = xT.reshape(6, N_CORES, SHARD).transpose(1, 0, 2).copy()

    bb0 = np.tile(np.asarray(b0, f32)[None, :], (128, 1))
    bb1 = np.tile(np.asarray(b1, f32)[None, :], (128, 1))
    bb2 = np.tile(np.asarray(b2, f32)[None, :], (128, 1))

    locT = np.asarray(loc, f32).reshape(N_GRAPHS * N_LOCS, 2).T.copy()
    ident = np.eye(128, dtype=f32)

    common = dict(
        xT=xT, rhs0=rhs0, rhs1=rhs1, rhs2=rhs2,
        bb0=bb0, bb1=bb1, bb2=bb2, cntinv=cntinv,
        locT=locT, wl1=np.asarray(Wl1, f32), bl1c=np.asarray(bl1, f32).reshape(-1, 1),
        wl2=np.asarray(Wl2, f32), bl2c=np.asarray(bl2, f32).reshape(-1, 1),
        ident=ident,
    )
    in_maps = []
    for c in range(N_CORES):
        m = dict(common)
        m["idx"] = idx_dev[c]
        m["rel"] = rel_dev[c]
        m["bpool"] = bp[c]
        m["xTo"] = xT_own[c]
        m["pdst"] = pdst_dev[c]
        in_maps.append(m)
    return in_maps, (K, K_nt)


def _build(Kinfo):
    K, K_nt = Kinfo
    _install_tile_patch()
    import concourse.bass as bass
    import concourse.mybir as mybir
    import concourse.tile as tile

    f32 = mybir.dt.float32
    i32 = mybir.dt.int32
    AF = mybir.ActivationFunctionType

    nc = bass.Bass(num_devices=N_CORES)

    inp = {}
    for name, shape, dt in [
        ("xT", [6, NPAD], f32), ("xTo", [6, SHARD], f32),
        ("idx", [NT, 128, K], i32), ("rel", [NT, 128, K], f32),
        ("pdst", [NT, K, 128, 128], mybir.dt.float8e4),
        ("bpool", [NT, 128, N_GRAPHS], f32), ("cntinv", [N_GRAPHS, 1], f32),
        ("rhs0", [6, MID + 6], f32), ("rhs1", [MID, MID + 6], f32),
        ("rhs2", [MID, GH + 2], f32),
        ("bb0", [128, MID], f32), ("bb1", [128, MID], f32), ("bb2", [128, GH], f32),
        ("locT", [2, N_GRAPHS * N_LOCS], f32), ("wl1", [2, MLPH], f32),
        ("bl1c", [MLPH, 1], f32), ("wl2", [MLPH, GH], f32), ("bl2c", [GH, 1], f32),
        ("ident", [128, 128], f32),
    ]:
        inp[name] = nc.dram_tensor(name, shape, dt, kind="ExternalInput")

    out = nc.dram_tensor("out", [N_GRAPHS, GH * 2], f32, kind="ExternalOutput")

    # tables
    T = [
        nc.dram_tensor("T0", [NPAD + 1, TW], f32, kind="Internal"),
        nc.dram_tensor("T1", [NPAD + 1, TW], f32, kind="Internal",
                       addr_space="Shared"),
        nc.dram_tensor("T2", [NPAD + 1, TW], f32, kind="Internal",
                       addr_space="Shared"),
    ]
    Tsh = [
        nc.dram_tensor("Tsh1", [SHARD, TW], f32, kind="Internal"),
        nc.dram_tensor("Tsh2", [SHARD, TW], f32, kind="Internal"),
    ]
    s_in = nc.dram_tensor("s_in", [N_GRAPHS, GH], f32, kind="Internal")
    s_out = nc.dram_tensor("s_out", [N_GRAPHS, GH], f32, kind="Internal",
                           addr_space="Shared")

    # per-layer config: (feat width F, heads H) — rhs width = F + H
    LCFG = [(MID, HEADS), (MID, HEADS), (GH, 1)]
    RG = [list(range(N_CORES))]

    with tile.TileContext(nc) as tc:
        with tc.tile_pool(name="const", bufs=1) as cp, \
             tc.tile_pool(name="sb", bufs=8) as sb, \
             tc.tile_pool(name="ed", bufs=1) as edp, \
             tc.tile_pool(name="mlp", bufs=1) as mlppool, \
             tc.tile_pool(name="ps", bufs=3, space="PSUM") as ps, \
             tc.tile_pool(name="pse", bufs=2, space="PSUM") as pse, \
             tc.tile_pool(name="psacc", bufs=2, space="PSUM") as psacc, \
             tc.tile_pool(name="pspool", bufs=1, space="PSUM") as pspool:

            ident = cp.tile([128, 128], f32)
            nc.sync.dma_start(ident[:], inp["ident"][:, :])
            iota_row = cp.tile([128, 128], f32)
            nc.gpsimd.iota(iota_row[:], pattern=[[1, 128]], base=0,
                        channel_multiplier=0,
                        allow_small_or_imprecise_dtypes=True)

            rhs_sb = []      # layer 0: [6, 135]
            t0r = cp.tile([6, MID + 6], f32, tag="rhs0")
            nc.sync.dma_start(t0r[:], inp["rhs0"][:, :])
            rhs_sb.append(t0r)
            rhs_a, rhs_b = {}, {}   # layers 1,2: split [128,W] + [1,W]
            for l, nm in [(1, "rhs1"), (2, "rhs2")]:
                w = inp[nm].shape[1]
                ta = cp.tile([128, w], f32, tag=f"rhsa{l}")
                nc.sync.dma_start(ta[:], inp[nm][0:128, :])
                tb = cp.tile([1, w], f32, tag=f"rhsb{l}")
                nc.sync.dma_start(tb[:], inp[nm][128:129, :])
                rhs_a[l], rhs_b[l] = ta, tb
            bb_sb = []
            for l, nm in enumerate(["bb0", "bb1", "bb2"]):
                t = cp.tile(list(inp[nm].shape), f32, tag=f"bb{l}")
                nc.sync.dma_start(t[:], inp[nm][:, :])
                bb_sb.append(t)

            # ed values for own shard, per layer: [128, NT, H]
            ed_all = [edp.tile([128, NT, 3], mybir.dt.bfloat16, tag=f"edall{l}", name=f"edall{l}") for l in range(3)]

            # ---------------- stage A: build T0 for all nodes ----------------
            for j in range(NPAD // 128):
                xs = sb.tile([6, 128], f32, tag="xs")
                nc.sync.dma_start(xs[:], inp["xT"][:, j * 128:(j + 1) * 128])
                ptab = ps.tile([128, MID + 6], f32, space="PSUM", tag="pscr")
                nc.tensor.matmul(ptab[:], lhsT=xs[:], rhs=rhs_sb[0][:],
                                 start=True, stop=True)
                stg = sb.tile([128, TW], f32, tag="stg0")
                nc.vector.memset(stg[:, MID + 6:], 0.0)
                nc.vector.tensor_copy(stg[:, :MID + 6], ptab[:])
                nc.sync.dma_start(T[0][j * 128:(j + 1) * 128, :], stg[:])
            zz = sb.tile([1, TW], f32, tag="zrow")
            nc.vector.memset(zz[:], 0.0)
            nc.vector.memset(zz[:, MID:MID + 3], -1000.0)
            nc.sync.dma_start(T[0][ZROW:ZROW + 1, :], zz[:])
            # ed0 for own shard (from per-core xTo input)
            for t in range(NT):
                xo = sb.tile([6, 128], f32, tag="xs")
                nc.sync.dma_start(xo[:], inp["xTo"][:, t * 128:(t + 1) * 128])
                pe0 = ps.tile([128, 6], f32, space="PSUM", tag="pscr")
                nc.tensor.matmul(pe0[:], lhsT=xo[:], rhs=rhs_sb[0][:, MID:MID + 6],
                                 start=True, stop=True)
                nc.vector.tensor_copy(ed_all[0][:, t, :], pe0[:, 3:6])

            # ---------------- loc MLP (independent) ----------------
            locT = mlppool.tile([2, N_GRAPHS * N_LOCS], f32)
            nc.sync.dma_start(locT[:], inp["locT"][:, :])
            wl1 = mlppool.tile([2, MLPH], f32)
            nc.sync.dma_start(wl1[:], inp["wl1"][:, :])
            bl1c = mlppool.tile([128, 2], f32)
            nc.sync.dma_start(bl1c[:], inp["bl1c"][:, 0:1].rearrange("(h p) o -> p (h o)", p=128))
            wl2a = mlppool.tile([128, GH], f32)
            nc.sync.dma_start(wl2a[:], inp["wl2"][0:128, :])
            wl2b = mlppool.tile([128, GH], f32)
            nc.sync.dma_start(wl2b[:], inp["wl2"][128:256, :])
            bl2c = mlppool.tile([GH, 1], f32)
            nc.sync.dma_start(bl2c[:], inp["bl2c"][:, :])
            mid_sb = [mlppool.tile([128, N_GRAPHS * N_LOCS], f32, tag=f"mid{h}", name=f"mid{h}")
                      for h in range(2)]
            CH = 400
            nch = (N_GRAPHS * N_LOCS) // CH
            for h in range(2):
                for c in range(nch):
                    pm = ps.tile([128, CH], f32, space="PSUM", tag="pscr")
                    nc.tensor.matmul(pm[:], lhsT=wl1[:, h * 128:(h + 1) * 128],
                                     rhs=locT[:, c * CH:(c + 1) * CH],
                                     start=True, stop=True)
                    nc.scalar.activation(mid_sb[h][:, c * CH:(c + 1) * CH], pm[:],
                                         AF.Tanh, bias=bl1c[:, h:h + 1])
            lpT = mlppool.tile([128, N_GRAPHS], f32)
            for c in range(nch):
                po = ps.tile([128, CH], f32, space="PSUM", tag="pscr")
                nc.tensor.matmul(po[:], lhsT=wl2a[:],
                                 rhs=mid_sb[0][:, c * CH:(c + 1) * CH],
                                 start=True, stop=False)
                nc.tensor.matmul(po[:], lhsT=wl2b[:],
                                 rhs=mid_sb[1][:, c * CH:(c + 1) * CH],
                                 start=False, stop=True)
                ng = CH // N_LOCS
                nc.vector.reduce_sum(
                    lpT[:, c * ng:(c + 1) * ng],
                    po[:].rearrange("p (g l) -> p g l", l=N_LOCS),
                    axis=mybir.AxisListType.X)
            lpT2 = mlppool.tile([128, N_GRAPHS], f32)
            nc.vector.tensor_scalar(lpT2[:], lpT[:], 1.0 / N_LOCS, bl2c[:],
                                    mybir.AluOpType.mult, mybir.AluOpType.add)
            plp = ps.tile([N_GRAPHS, 128], f32, space="PSUM", tag="pscr")
            nc.tensor.matmul(plp[:], lhsT=lpT2[:], rhs=ident[:],
                             start=True, stop=True)
            out_sb = mlppool.tile([N_GRAPHS, 2 * GH], f32)
            nc.vector.tensor_copy(out_sb[:, GH:], plp[:])

            # ---------------- GAT layers ----------------
            psum_S = pspool.tile([N_GRAPHS, GH], f32, space="PSUM")

            for l in range(3):
                F, H = LCFG[l]
                RW = F + H                       # rhs/scatter width
                tab = T[l]
                for nt in range(NT):
                    idxs = sb.tile([128, K], i32, tag="idxs")
                    nc.sync.dma_start(idxs[:], inp["idx"][nt, :, :])
                    rels = sb.tile([128, K], f32, tag="rels")
                    nc.sync.dma_start(rels[:], inp["rel"][nt, :, :])
                    acc = psacc.tile([128, RW], f32, space="PSUM", tag="acc")
                    Kt = K_nt[nt]
                    for t in range(Kt):
                        g = sb.tile([128, TW], f32, tag="g")
                        nc.gpsimd.indirect_dma_start(
                            out=g[:], out_offset=None, in_=tab[:, :],
                            in_offset=bass.IndirectOffsetOnAxis(
                                ap=idxs[:, t:t + 1], axis=0))
                        pedge = sb.tile([128, 128], f32, tag="pedge")
                        nc.vector.tensor_tensor(
                            out=pedge[:], in0=rels[:, t:t + 1].to_broadcast([128, 128]),
                            in1=iota_row[:], op=mybir.AluOpType.is_equal)
                        pdst = sb.tile([128, 128], mybir.dt.float8e4, tag="pdst")
                        nc.sync.dma_start(pdst[:], inp["pdst"][nt, t, :, :])
                        pede = pse.tile([128, 3], f32, space="PSUM", tag="pede")
                        nc.tensor.matmul(pede[:, :H], lhsT=pdst[:],
                                         rhs=ed_all[l][:, nt, :H],
                                         start=True, stop=True)
                        e = sb.tile([128, 3], f32, tag="e")
                        nc.vector.tensor_add(e[:, :H], g[:, F:F + H], pede[:, :H])
                        lk = sb.tile([128, 3], f32, tag="lk")
                        nc.scalar.activation(lk[:, :H], e[:, :H], AF.Prelu, alpha=NEG)
                        w = sb.tile([128, 3], f32, tag="w")
                        nc.scalar.activation(w[:, :H], lk[:, :H], AF.Exp)
                        rhs_t = sb.tile([128, RW], f32, tag="rhs_t")
                        oph = F // H
                        for hh in range(H):
                            nc.vector.tensor_scalar_mul(
                                rhs_t[:, hh * oph:(hh + 1) * oph],
                                g[:, hh * oph:(hh + 1) * oph], w[:, hh:hh + 1])
                        nc.vector.tensor_copy(rhs_t[:, F:F + H], w[:, :H])
                        nc.tensor.matmul(acc[:], lhsT=pedge[:], rhs=rhs_t[:],
                                         start=(t == 0), stop=(t == Kt - 1))
                    # epilogue for this node-tile
                    zc = sb.tile([128, 3], f32, tag="zc")
                    nc.vector.tensor_scalar_max(zc[:, :H], acc[:, F:F + H], 1e-30)
                    zr = sb.tile([128, 3], f32, tag="zr")
                    nc.vector.reciprocal(zr[:, :H], zc[:, :H])
                    u = sb.tile([128, F], f32, tag="u")
                    oph = F // H
                    for hh in range(H):
                        nc.vector.tensor_scalar_mul(
                            u[:, hh * oph:(hh + 1) * oph],
                            acc[:, hh * oph:(hh + 1) * oph], zr[:, hh:hh + 1])
                    ob = sb.tile([128, F], f32, tag="ob")
                    nc.vector.tensor_add(ob[:], u[:], bb_sb[l][:, :F])
                    if l < 2:
                        hin = sb.tile([128, F], f32, tag="hin")
                        nc.scalar.activation(hin[:], ob[:], AF.Prelu, alpha=NEG)
                        # transpose hin -> [F, 128] in two pieces
                        ph1 = ps.tile([128, 128], f32, space="PSUM", tag="pscr")
                        nc.tensor.matmul(ph1[:], lhsT=hin[:, 0:128], rhs=ident[:],
                                         start=True, stop=True)
                        hTa = sb.tile([128, 128], f32, tag="hTa")
                        nc.vector.tensor_copy(hTa[:], ph1[:])
                        ph2 = ps.tile([1, 128], f32, space="PSUM", tag="pscr")
                        nc.tensor.matmul(ph2[:], lhsT=hin[:, 128:129], rhs=ident[:],
                                         start=True, stop=True)
                        hTb = sb.tile([1, 128], f32, tag="hTb")
                        nc.vector.tensor_copy(hTb[:], ph2[:])
                        nF2, nH2 = LCFG[l + 1]
                        ptab = ps.tile([128, nF2 + 2 * nH2], f32, space="PSUM",
                                       tag="pscr")
                        nc.tensor.matmul(ptab[:], lhsT=hTa[:],
                                         rhs=rhs_a[l + 1][:],
                                         start=True, stop=False)
                        nc.tensor.matmul(ptab[:], lhsT=hTb[:],
                                         rhs=rhs_b[l + 1][:],
                                         start=False, stop=True)
                        stg = sb.tile([128, TW], f32, tag="stg")
                        nc.vector.memset(stg[:, nF2 + 2 * nH2:], 0.0)
                        nc.vector.tensor_copy(stg[:, :nF2 + 2 * nH2], ptab[:])
                        nc.sync.dma_start(
                            Tsh[l][nt * 128:(nt + 1) * 128, :], stg[:])
                        nc.vector.tensor_copy(
                            ed_all[l + 1][:, nt, :nH2],
                            ptab[:, nF2 + nH2:nF2 + 2 * nH2])
                    else:
                        bpt = sb.tile([128, N_GRAPHS], f32, tag="bpt")
                        nc.sync.dma_start(bpt[:], inp["bpool"][nt, :, :])
                        nc.tensor.matmul(psum_S[:], lhsT=bpt[:], rhs=ob[:],
                                         start=(nt == 0), stop=(nt == NT - 1))
                if l < 2:
                    # zero row of next table, then replicate shards
                    zz2 = sb.tile([1, TW], f32, tag="zrow")
                    nc.vector.memset(zz2[:], 0.0)
                    nF2, nH2 = LCFG[l + 1]
                    nc.vector.memset(zz2[:, nF2:nF2 + nH2], -1000.0)
                    nc.sync.dma_start(T[l + 1][ZROW:ZROW + 1, :], zz2[:])
                    nc.gpsimd.collective_compute(
                        "AllGather", mybir.AluOpType.bypass, replica_groups=RG,
                        ins=[Tsh[l][:, :]], outs=[T[l + 1][0:NPAD, :]])

            # pooling: AllReduce of per-shard sums, then divide by counts
            ssb = sb.tile([N_GRAPHS, GH], f32, tag="ssb")
            nc.vector.tensor_copy(ssb[:], psum_S[:])
            nc.sync.dma_start(s_in[:, :], ssb[:])
            nc.gpsimd.collective_compute(
                "AllReduce", mybir.AluOpType.add, replica_groups=RG,
                ins=[s_in[:, :]], outs=[s_out[:, :]])
            sfull = sb.tile([N_GRAPHS, GH], f32, tag="sfull")
            nc.sync.dma_start(sfull[:], s_out[:, :])
            civ = sb.tile([N_GRAPHS, 1], f32, tag="civ")
            nc.sync.dma_start(civ[:], inp["cntinv"][:, :])
            nc.vector.tensor_scalar_mul(out_sb[:, 0:GH], sfull[:], civ[:])
            nc.sync.dma_start(out[:, :], out_sb[:])

    return nc


def kernel(**inputs):
    key = "k"
    in_maps, Kinfo = _host_prep(**inputs)
    if key not in _CACHE or _CACHE[key][1] != Kinfo:
        nc = _build(Kinfo)
        _CACHE[key] = (nc, Kinfo)
    nc = _CACHE[key][0]
    from concourse.bass_utils import run_bass_kernel_spmd
    res = run_bass_kernel_spmd(nc, in_maps, core_ids=list(range(N_CORES)))
    return np.asarray(res.results[0]["out"])
